# revision 27
# baseline (speedup 1.0000x reference)
"""Trainium2 Bass kernel for nn_LongformerClassifier (sparse_attention).

Strategy (validated against the reference in numpy first):
  - The model output is only the CLS-token logits (1, 50), so layer 2
    collapses to: global attention of token 0 over all x1 + one-row FFN.
  - Launch 1 (8 cores, sequence-parallel, 512 own tokens + 256 halo each
    side): embeddings + LN, full layer-1 (sliding-window + global-column
    attention, FFN), plus per-core partial softmax stats for layer-1
    global attention of token 0.
  - Host: reduce the tiny per-core stats -> x1[0] row (couple of matvecs).
  - Launch 2 (8 cores): per-core partial stats for layer-2 global
    attention of token 0 over x1.
  - Host: reduce -> layer-2 row FFN -> classifier -> logits.

  All activations are kept feature-major ([feature partitions, token
  free dim]) so no on-device transposes are needed anywhere; softmax is
  computed key-major without max-subtraction (scores are O(1) here) and
  partition-dim reductions are done with ones-vector matmuls.
  Matmuls run in bf16 (fp32 PSUM accumulation).
"""
import sys
import numpy as np
import ml_dtypes

sys.path.insert(0, "/opt/trn_rl_repo")

V, S, D, H, NUM_LABELS = 50265, 4096, 768, 12, 50
HD, C, FF = 64, 256, 3072
NC_CORES = 8
OWN = S // NC_CORES        # 512
EXT = OWN + 2 * C          # 1024 (token 0 appended as column EXT)
SCALE = 1.0 / np.sqrt(HD)
KT = D // 128              # 6 feature partition-tiles
MT_FF = FF // 128          # 24
BF = ml_dtypes.bfloat16

_cache = {}


# ----------------------------------------------------------------- bass build
def _mk(nc_mod, bacc_mod):
    pass


def _build_launch1():
    import concourse.bacc as bacc
    import concourse.mybir as mybir
    import concourse.tile as tile

    dt = mybir.dt
    AF = mybir.ActivationFunctionType
    ALU = mybir.AluOpType
    nc = bacc.Bacc("TRN2", target_bir_lowering=False, debug=False,
                   num_devices=NC_CORES)

    def din(name, shape, dtype=dt.float32):
        return nc.dram_tensor(name, shape, dtype, kind="ExternalInput").ap()

    def dout(name, shape, dtype=dt.float32):
        return nc.dram_tensor(name, shape, dtype, kind="ExternalOutput").ap()

    NEXT = EXT + 1  # 1025
    embT = din("embT", [KT, 128, NEXT])
    g_emb = din("g_emb", [KT, 128, 1]); b_emb = din("b_emb", [KT, 128, 1])
    Wq = din("Wq", [KT, 128, D], dt.bfloat16); bq = din("bq", [KT, 128, 1])
    Wk = din("Wk", [KT, 128, D], dt.bfloat16)
    Wv = din("Wv", [KT, 128, D], dt.bfloat16)
    Wkg = din("Wkg", [KT, 128, D], dt.bfloat16)
    Wvg = din("Wvg", [KT, 128, D], dt.bfloat16)
    Wqg = din("Wqg", [KT, 128, D], dt.bfloat16); bqg = din("bqg", [KT, 128, 1])
    Wo = din("Wo", [KT, 128, D], dt.bfloat16); bo = din("bo", [KT, 128, 1])
    g1 = din("g1", [KT, 128, 1]); be1 = din("be1", [KT, 128, 1])
    W1 = din("W1", [KT, 128, FF], dt.bfloat16); b1 = din("b1", [MT_FF, 128, 1])
    W2 = din("W2", [MT_FF, 128, D], dt.bfloat16); b2 = din("b2", [KT, 128, 1])
    g2 = din("g2", [KT, 128, 1]); be2 = din("be2", [KT, 128, 1])
    maskT = din("maskT", [2, 6, 128, 256], dt.bfloat16)
    gmask = din("gmask", [4, 128, 1])

    x1T_out = dout("x1T", [KT, 128, OWN])
    gstats_out = dout("gstats", [65, H])
    x0row_out = dout("x0row", [KT, 128, 1])

    with tile.TileContext(nc) as tc:
        import contextlib
        with contextlib.ExitStack() as ctx:
            const = ctx.enter_context(tc.tile_pool(name="const", bufs=1))
            ones_bf = const.tile([128, 1], dt.bfloat16, tag="ones_bf",
                                 name="ones_bf")
            nc.vector.memset(ones_bf[:], 1.0)
            eps_t = const.tile([1, 1], dt.float32, tag="eps_t", name="eps_t")
            nc.vector.memset(eps_t[:], 1e-5)

            perm = ctx.enter_context(tc.tile_pool(name="perm", bufs=1))
            trans = ctx.enter_context(tc.tile_pool(name="trans", bufs=2))
            wpool = ctx.enter_context(tc.tile_pool(name="wpool", bufs=1))

            def load_cols(ap, n, dtype=dt.float32):
                ts = []
                for i in range(n):
                    t = perm.tile([128, 1], dtype,
                                  tag=f"pp{ap.tensor.name}{i}",
                                  name=f"pp{ap.tensor.name}{i}")
                    nc.sync.dma_start(t[:], ap[i])
                    ts.append(t)
                return ts

            bq_t = load_cols(bq, KT); bqg_t = load_cols(bqg, KT)
            bo_t = load_cols(bo, KT); b1_t = load_cols(b1, MT_FF)
            b2_t = load_cols(b2, KT)
            g_emb_t = load_cols(g_emb, KT); b_emb_t = load_cols(b_emb, KT)
            g1_t = load_cols(g1, KT); be1_t = load_cols(be1, KT)
            g2_t = load_cols(g2, KT); be2_t = load_cols(be2, KT)
            gmask_t = load_cols(gmask, 4)

            # weight slots: small double-buffered for 768-wide matrices,
            # big single-buffered only for W1
            def load_w_shared(ap, ntiles, cols, big=False):
                ts = []
                for k in range(ntiles):
                    if big:
                        t = wpool.tile([128, cols], dt.bfloat16, tag=f"wld{k}",
                                       name=f"wld{k}", bufs=1)
                    else:
                        t = wpool.tile([128, cols], dt.bfloat16, tag=f"wsm{k}",
                                       name=f"wsm{k}", bufs=2)
                    nc.sync.dma_start(t[:], ap[k])
                    ts.append(t)
                return ts

            # ------------- feature-major layernorm (in-place capable) -------
            def ln_feat(xf32, ncols, gam, bet, out_tags, want_bf, bf_tags):
                nspl = [(o, min(512, ncols - o)) for o in range(0, ncols, 512)]
                pre_bf = []
                for k in range(KT):
                    pb = perm.tile([128, ncols], dt.bfloat16, tag=f"preb{k}",
                                   name=f"preb{k}")
                    nc.vector.tensor_copy(pb[:], xf32[k][:])
                    pre_bf.append(pb)
                srow = trans.tile([1, NEXT], dt.float32, tag="ln_srow",
                                  name="ln_srow", bufs=1)
                qrow = trans.tile([1, NEXT], dt.float32, tag="ln_qrow",
                                  name="ln_qrow", bufs=1)
                with tc.tile_pool(name="lnps", bufs=2, space="PSUM") as psm:
                    for (o, n) in nspl:
                        ps = psm.tile([1, 512], dt.float32, tag="ln_ps",
                                      name="ln_ps")
                        ps2 = psm.tile([1, 512], dt.float32, tag="ln_ps2",
                                       name="ln_ps2")
                        for k in range(KT):
                            nc.tensor.matmul(ps[:, :n], ones_bf[:],
                                             pre_bf[k][:, o:o + n],
                                             start=(k == 0),
                                             stop=(k == KT - 1))
                            sqc = trans.tile([128, 512], dt.bfloat16,
                                             tag="lnsqc", name="lnsqc",
                                             bufs=2)
                            nc.vector.tensor_mul(sqc[:, :n],
                                                 pre_bf[k][:, o:o + n],
                                                 pre_bf[k][:, o:o + n])
                            nc.tensor.matmul(ps2[:, :n], ones_bf[:],
                                             sqc[:, :n],
                                             start=(k == 0),
                                             stop=(k == KT - 1))
                        nc.vector.tensor_scalar_mul(srow[:, o:o + n],
                                                    ps[:, :n], -1.0 / D)
                        nc.vector.tensor_scalar_mul(qrow[:, o:o + n],
                                                    ps2[:, :n], 1.0 / D)
                m2 = trans.tile([1, NEXT], dt.float32, tag="ln_m2",
                                name="ln_m2", bufs=1)
                rst = trans.tile([1, NEXT], dt.float32, tag="ln_rst",
                                name="ln_rst", bufs=1)
                nc.vector.tensor_mul(m2[:, :ncols], srow[:, :ncols],
                                     srow[:, :ncols])
                nc.vector.tensor_sub(qrow[:, :ncols], qrow[:, :ncols],
                                     m2[:, :ncols])
                nc.scalar.activation(m2[:, :ncols], qrow[:, :ncols],
                                     AF.Sqrt, bias=eps_t[:])
                nc.vector.reciprocal(rst[:, :ncols], m2[:, :ncols])
                nm_b = trans.tile([128, NEXT], dt.float32, tag="ln_nmb",
                                  name="ln_nmb", bufs=1)
                rs_b = trans.tile([128, NEXT], dt.float32, tag="ln_rsb",
                                  name="ln_rsb", bufs=1)
                nc.gpsimd.partition_broadcast(nm_b[:, :ncols], srow[:, :ncols])
                nc.gpsimd.partition_broadcast(rs_b[:, :ncols], rst[:, :ncols])
                out_f32, out_bf = [], []
                for k in range(KT):
                    xc = trans.tile([128, NEXT], dt.float32, tag="ln_xc",
                                    name="ln_xc", bufs=1)
                    nc.vector.tensor_add(xc[:, :ncols], xf32[k][:],
                                         nm_b[:, :ncols])
                    nc.vector.tensor_mul(xc[:, :ncols], xc[:, :ncols],
                                         rs_b[:, :ncols])
                    of = perm.tile([128, ncols], dt.float32, tag=out_tags[k],
                                   name=out_tags[k])
                    nc.vector.tensor_scalar(of[:], xc[:, :ncols], gam[k][:],
                                            bet[k][:], ALU.mult, ALU.add)
                    out_f32.append(of)
                    if want_bf:
                        ob = perm.tile([128, ncols], dt.bfloat16,
                                       tag=bf_tags[k], name=bf_tags[k])
                        nc.vector.tensor_copy(ob[:], of[:])
                        out_bf.append(ob)
                return out_f32, out_bf

            # ---------------- phase A: embeddings + emb LN -------------------
            xpre = []
            for k in range(KT):
                xz = perm.tile([128, NEXT], dt.float32, tag=f"xz{k}",
                               name=f"xz{k}")
                nc.sync.dma_start(xz[:, 0:513], embT[k][:, 0:513])
                nc.sync.dma_start(xz[:, 513:NEXT], embT[k][:, 513:NEXT])
                xpre.append(xz)
            x0T, x0T_bf = ln_feat(xpre, NEXT, g_emb_t, b_emb_t,
                                  [f"xz{k}" for k in range(KT)],
                                  True, [f"preb{k}" for k in range(KT)])
            for k in range(KT):
                nc.sync.dma_start(x0row_out[k], x0T[k][:, EXT:EXT + 1])

            # ---------------- phase B: projections ---------------------------
            def proj_fm(col0, ncols, bias_ts, tag, psm, wtiles):
                outs = []
                nspl = [(o, min(512, ncols - o)) for o in range(0, ncols, 512)]
                for m in range(KT):
                    out = perm.tile([128, ncols], dt.bfloat16, tag=f"{tag}{m}",
                                    name=f"{tag}{m}")
                    for (o, n) in nspl:
                        ps = psm.tile([128, 512], dt.float32, tag="projps",
                                      name="projps")
                        for k in range(KT):
                            nc.tensor.matmul(
                                ps[:, :n],
                                wtiles[k][:, m * 128:(m + 1) * 128],
                                x0T_bf[k][:, col0 + o:col0 + o + n],
                                start=(k == 0), stop=(k == KT - 1))
                        if bias_ts is None:
                            nc.vector.tensor_copy(out[:, o:o + n], ps[:, :n])
                        else:
                            nc.scalar.activation(out[:, o:o + n], ps[:, :n],
                                                 AF.Identity,
                                                 bias=bias_ts[m][:])
                    outs.append(out)
                return outs

            with tc.tile_pool(name="psB", bufs=4, space="PSUM") as psB:
                Wk_t = load_w_shared(Wk, KT, D)
                KT_bf = proj_fm(0, NEXT, None, "ad", psB, Wk_t)
                Wq_t = load_w_shared(Wq, KT, D)
                QT_bf = proj_fm(C, OWN, bq_t, "QTb", psB, Wq_t)
                Wkg_t = load_w_shared(Wkg, KT, D)
                kgT_bf = proj_fm(C, OWN, None, "kgTb", psB, Wkg_t)
                Wqg_t = load_w_shared(Wqg, KT, D)
                qgT_bf = proj_fm(EXT, 1, bqg_t, "qgTb", psB, Wqg_t)

            with tc.tile_pool(name="psV", bufs=1, space="PSUM") as psV:
                Wv_t = load_w_shared(Wv, KT, D)
                V_bf = []
                for t in range(8):
                    ps = psV.tile([128, D], dt.float32, tag="vps",
                                  name="vps", bufs=2)
                    for (o, n) in [(0, 512), (512, 256)]:
                        for k in range(KT):
                            nc.tensor.matmul(
                                ps[:, o:o + n],
                                x0T_bf[k][:, t * 128:(t + 1) * 128],
                                Wv_t[k][:, o:o + n],
                                start=(k == 0), stop=(k == KT - 1))
                    vb = perm.tile([128, D], dt.bfloat16, tag=f"Vb{t}",
                                   name=f"Vb{t}")
                    nc.vector.tensor_copy(vb[:], ps[:])
                    V_bf.append(vb)
                v0_bf = perm.tile([1, D], dt.bfloat16, tag="v0b", name="v0b")
                ps0 = psV.tile([1, D], dt.float32, tag="v0ps", name="v0ps",
                               bufs=1)
                for (o, n) in [(0, 512), (512, 256)]:
                    for k in range(KT):
                        nc.tensor.matmul(ps0[:, o:o + n],
                                         x0T_bf[k][:, EXT:EXT + 1],
                                         Wv_t[k][:, o:o + n],
                                         start=(k == 0), stop=(k == KT - 1))
                nc.vector.tensor_copy(v0_bf[:], ps0[:])
                Wvg_t = load_w_shared(Wvg, KT, D)
                vg_bf = []
                for t in range(4):
                    ps = psV.tile([128, D], dt.float32, tag="vps",
                                  name="vps", bufs=2)
                    for (o, n) in [(0, 512), (512, 256)]:
                        for k in range(KT):
                            nc.tensor.matmul(
                                ps[:, o:o + n],
                                x0T_bf[k][:, C + t * 128:C + (t + 1) * 128],
                                Wvg_t[k][:, o:o + n],
                                start=(k == 0), stop=(k == KT - 1))
                    vb = perm.tile([128, D], dt.bfloat16, tag=f"vgb{t}",
                                   name=f"vgb{t}")
                    nc.vector.tensor_copy(vb[:], ps[:])
                    vg_bf.append(vb)

            mk_t = [[None] * 6 for _ in range(2)]
            for lt in range(2):
                for i in range(6):
                    mt_ = perm.tile([128, 256], dt.bfloat16,
                                    tag=f"mk{lt}_{i}", name=f"mk{lt}_{i}")
                    nc.sync.dma_start(mt_[:], maskT[lt, i])
                    mk_t[lt][i] = mt_

            # ---------------- phase F: layer-1 global stats ------------------
            gs_sb = perm.tile([65, H], dt.float32, tag="gs", name="gs")
            with tc.tile_pool(name="psF", bufs=2, space="PSUM") as psF:
                for pt in range(KT):
                    hA, hB = 2 * pt, 2 * pt + 1
                    qg2c = trans.tile([128, 2], dt.bfloat16, tag="qg2c",
                                      name="qg2c", bufs=2)
                    nc.vector.memset(qg2c[:], 0.0)
                    nc.vector.tensor_copy(qg2c[0:64, 0:1], qgT_bf[pt][0:64, :])
                    nc.vector.tensor_copy(qg2c[64:128, 1:2],
                                          qgT_bf[pt][64:128, :])
                    ps_acc = psF.tile([128, 2], dt.float32, tag="facc",
                                      name="facc")
                    ps_sum = psF.tile([1, 2], dt.float32, tag="fsum",
                                      name="fsum")
                    for j in range(4):
                        ps_s = psF.tile([128, 2], dt.float32, tag="fsgf",
                                        name="fsgf")
                        nc.tensor.matmul(
                            ps_s[:],
                            kgT_bf[pt][:, j * 128:(j + 1) * 128], qg2c[:])
                        e = trans.tile([128, 2], dt.float32, tag="fe",
                                       name="fe")
                        nc.scalar.activation(e[:], ps_s[:], AF.Exp)
                        eb = trans.tile([128, 2], dt.bfloat16, tag="feb",
                                        name="feb")
                        nc.vector.tensor_scalar_mul(eb[:], e[:],
                                                    gmask_t[j][:])
                        nc.tensor.matmul(ps_acc[:],
                                         vg_bf[j][:, pt * 128:(pt + 1) * 128],
                                         eb[:], start=(j == 0), stop=(j == 3))
                        nc.tensor.matmul(ps_sum[:], ones_bf[:], eb[:],
                                         start=(j == 0), stop=(j == 3))
                    nc.vector.tensor_copy(gs_sb[0:64, hA:hA + 1],
                                          ps_acc[0:64, 0:1])
                    nc.vector.tensor_copy(gs_sb[0:64, hB:hB + 1],
                                          ps_acc[64:128, 1:2])
                    nc.vector.tensor_copy(gs_sb[64:65, hA:hA + 1],
                                          ps_sum[0:1, 0:1])
                    nc.vector.tensor_copy(gs_sb[64:65, hB:hB + 1],
                                          ps_sum[0:1, 1:2])
            nc.sync.dma_start(gstats_out[:], gs_sb[:])

            # ---------------- phase C: windowed attention --------------------
            attnT_bf = []
            for k in range(KT):
                at = perm.tile([128, OWN], dt.bfloat16, tag=f"at{k}",
                               name=f"at{k}")
                attnT_bf.append(at)
            with (tc.tile_pool(name="psS", bufs=1, space="PSUM") as psS,
                  tc.tile_pool(name="psG", bufs=1, space="PSUM") as psG,
                  tc.tile_pool(name="psO", bufs=2, space="PSUM") as psO):
                def stage1(h, lt):
                    pt, ro = h // 2, (h % 2) * 64
                    qs = QT_bf[pt][ro:ro + 64, lt * 256:(lt + 1) * 256]
                    es = trans.tile([128, 6, 256], dt.bfloat16, tag="es",
                                    name="es", bufs=2)
                    for w in range(2):
                        ps_s = psS.tile([128, 3, 256], dt.float32,
                                        tag="ps_s", name="ps_s", bufs=2)
                        for i3 in range(3):
                            i = w * 3 + i3
                            nc.tensor.matmul(
                                ps_s[:, i3, :],
                                KT_bf[pt][ro:ro + 64,
                                          lt * 256 + i * 128:
                                          lt * 256 + (i + 1) * 128],
                                qs)
                            nc.scalar.activation(es[:, i, :],
                                                 ps_s[:, i3, :], AF.Exp)
                            nc.vector.tensor_mul(es[:, i, :], es[:, i, :],
                                                 mk_t[lt][i][:])
                    ps_sg = psG.tile([1, 256], dt.float32, tag="ps_sg",
                                     name="ps_sg")
                    nc.tensor.matmul(ps_sg[:],
                                     KT_bf[pt][ro:ro + 64, EXT:EXT + 1], qs)
                    esg = trans.tile([1, 256], dt.bfloat16, tag="esg",
                                     name="esg", bufs=2)
                    nc.scalar.activation(esg[:], ps_sg[:], AF.Exp)
                    return es, esg

                def stage2(h, lt, es, esg):
                    pt, ro = h // 2, (h % 2) * 64
                    ps_den = psG.tile([1, 256], dt.float32, tag="ps_den",
                                      name="ps_den")
                    for i in range(6):
                        nc.tensor.matmul(ps_den[:], ones_bf[:],
                                         es[:, i, :], start=(i == 0),
                                         stop=False)
                    nc.tensor.matmul(ps_den[:], ones_bf[0:1, :], esg[:],
                                     start=False, stop=True)
                    rrow = trans.tile([1, 256], dt.float32, tag="rrow",
                                      name="rrow", bufs=2)
                    nc.vector.reciprocal(rrow[:], ps_den[:])
                    rb = trans.tile([64, 256], dt.float32, tag="rb",
                                    name="rb", bufs=2)
                    nc.gpsimd.partition_broadcast(rb[:], rrow[:])
                    ps_o = psO.tile([64, 256], dt.float32, tag="ps_o",
                                    name="ps_o")
                    for i in range(6):
                        nc.tensor.matmul(
                            ps_o[:],
                            V_bf[lt * 2 + i][:, h * 64:(h + 1) * 64],
                            es[:, i, :], start=(i == 0), stop=False)
                    nc.tensor.matmul(ps_o[:],
                                     v0_bf[:, h * 64:(h + 1) * 64],
                                     esg[:], start=False, stop=True)
                    nc.vector.tensor_mul(
                        attnT_bf[pt][ro:ro + 64, lt * 256:(lt + 1) * 256],
                        ps_o[:], rb[:])

                pend = None
                for pt in range(KT):
                    for lt in range(2):
                        for h in (2 * pt, 2 * pt + 1):
                            cur = stage1(h, lt)
                            if pend is not None:
                                stage2(*pend)
                            pend = (h, lt, *cur)
                stage2(*pend)

            # ---------------- phase D: attn proj + residual + LN1 ------------
            with tc.tile_pool(name="psD", bufs=4, space="PSUM") as psD:
                Wo_t = load_w_shared(Wo, KT, D)
                apre = []
                for m in range(KT):
                    ps = psD.tile([128, OWN], dt.float32, tag="dps",
                                  name="dps")
                    for k in range(KT):
                        nc.tensor.matmul(ps[:],
                                         Wo_t[k][:, m * 128:(m + 1) * 128],
                                         attnT_bf[k][:],
                                         start=(k == 0), stop=(k == KT - 1))
                    asb = trans.tile([128, OWN], dt.float32, tag="asb",
                                     name="asb")
                    nc.scalar.activation(asb[:], ps[:], AF.Identity,
                                         bias=bo_t[m][:])
                    ad = perm.tile([128, OWN], dt.float32, tag=f"ad{m}",
                                   name=f"ad{m}")
                    nc.vector.tensor_add(ad[:], asb[:], x0T[m][:, C:C + OWN])
                    apre.append(ad)
            xmT, xmT_bf = ln_feat(apre, OWN, g1_t, be1_t,
                                  [f"ad{k}" for k in range(KT)],
                                  True, [f"Vb{k}" for k in range(KT)])

            # ---------------- phase E: FFN + residual + LN2 ------------------
            with tc.tile_pool(name="psE", bufs=1, space="PSUM") as psE:
                W1_t = load_w_shared(W1, KT, FF, big=True)
                yps = [psE.tile([128, OWN], dt.float32, tag=f"yps{m}",
                                name=f"yps{m}") for m in range(KT)]
                ypre = []
                for m in range(MT_FF):
                    ps = psE.tile([128, OWN], dt.float32, tag="w1ps",
                                  name="w1ps", bufs=2)
                    for k in range(KT):
                        nc.tensor.matmul(ps[:],
                                         W1_t[k][:, m * 128:(m + 1) * 128],
                                         xmT_bf[k][:],
                                         start=(k == 0), stop=(k == KT - 1))
                    ht = trans.tile([128, OWN], dt.bfloat16, tag="hT",
                                    name="hT", bufs=3)
                    nc.scalar.activation(ht[:], ps[:], AF.Gelu_apprx_tanh,
                                         bias=b1_t[m][:])
                    w2t = trans.tile([128, D], dt.bfloat16, tag="w2ld",
                                     name="w2ld", bufs=3)
                    nc.sync.dma_start(w2t[:], W2[m])
                    for mo in range(KT):
                        nc.tensor.matmul(yps[mo][:],
                                         w2t[:, mo * 128:(mo + 1) * 128],
                                         ht[:], start=(m == 0),
                                         stop=(m == MT_FF - 1))
                for m in range(KT):
                    ysb = trans.tile([128, OWN], dt.float32, tag="ysb",
                                     name="ysb")
                    nc.scalar.activation(ysb[:], yps[m][:], AF.Identity,
                                         bias=b2_t[m][:])
                    yz = perm.tile([128, OWN], dt.float32, tag=f"xz{m}",
                                   name=f"xz{m}")
                    nc.vector.tensor_add(yz[:], ysb[:], xmT[m][:])
                    ypre.append(yz)
            x1T, _ = ln_feat(ypre, OWN, g2_t, be2_t,
                             [f"xz{k}" for k in range(KT)], False, None)
            for k in range(KT):
                nc.sync.dma_start(x1T_out[k], x1T[k][:])

    nc.compile()
    return nc


def _build_launch2():
    import concourse.bacc as bacc
    import concourse.mybir as mybir
    import concourse.tile as tile

    dt = mybir.dt
    AF = mybir.ActivationFunctionType
    nc = bacc.Bacc("TRN2", target_bir_lowering=False, debug=False,
                   num_devices=NC_CORES)

    def din(name, shape, dtype=dt.float32):
        return nc.dram_tensor(name, shape, dtype, kind="ExternalInput").ap()

    x1T = din("x1T", [KT, 128, OWN], dt.bfloat16)
    Wkg = din("Wkg", [KT, 128, D], dt.bfloat16)
    Wvg = din("Wvg", [KT, 128, D], dt.bfloat16)
    qgT = din("qgT", [KT, 128, 2], dt.bfloat16)   # zero-padded head-pair cols
    gmask = din("gmask", [4, 128, 1])
    gstats_out = nc.dram_tensor("gstats", [65, H], dt.float32,
                                kind="ExternalOutput").ap()

    with tile.TileContext(nc) as tc:
        import contextlib
        with contextlib.ExitStack() as ctx:
            pool = ctx.enter_context(tc.tile_pool(name="pool", bufs=1))
            const = ctx.enter_context(tc.tile_pool(name="const", bufs=1))
            ones_bf = const.tile([128, 1], dt.bfloat16, tag="ones_bf",
                                 name="ones_bf")
            nc.vector.memset(ones_bf[:], 1.0)
            x1_t, qg_t, Wkg_t, Wvg_t = [], [], [], []
            for k in range(KT):
                t = pool.tile([128, OWN], dt.bfloat16, tag=f"x1{k}",
                              name=f"x1{k}")
                nc.sync.dma_start(t[:], x1T[k])
                x1_t.append(t)
                t = pool.tile([128, D], dt.bfloat16, tag=f"Wkg{k}",
                              name=f"Wkg{k}")
                nc.sync.dma_start(t[:], Wkg[k])
                Wkg_t.append(t)
            for k in range(KT):
                t = pool.tile([128, 2], dt.bfloat16, tag=f"qg{k}",
                              name=f"qg{k}")
                nc.sync.dma_start(t[:], qgT[k])
                qg_t.append(t)
                t = pool.tile([128, D], dt.bfloat16, tag=f"Wvg{k}",
                              name=f"Wvg{k}")
                nc.sync.dma_start(t[:], Wvg[k])
                Wvg_t.append(t)
            gm_t = []
            for j in range(4):
                t = pool.tile([128, 1], dt.float32, tag=f"gm{j}",
                              name=f"gm{j}")
                nc.sync.dma_start(t[:], gmask[j])
                gm_t.append(t)

            with tc.tile_pool(name="ps", bufs=1, space="PSUM") as psm:
                kgT_bf = []
                for m in range(KT):
                    ps = psm.tile([128, OWN], dt.float32, tag="kps",
                                  name="kps", bufs=2)
                    for k in range(KT):
                        nc.tensor.matmul(ps[:],
                                         Wkg_t[k][:, m * 128:(m + 1) * 128],
                                         x1_t[k][:],
                                         start=(k == 0), stop=(k == KT - 1))
                    kg = pool.tile([128, OWN], dt.bfloat16, tag=f"kg{m}",
                                   name=f"kg{m}")
                    nc.vector.tensor_copy(kg[:], ps[:])
                    kgT_bf.append(kg)
                vg_bf = []
                for t in range(4):
                    ps = psm.tile([128, D], dt.float32, tag="vps",
                                  name="vps", bufs=1)
                    for (o, n) in [(0, 512), (512, 256)]:
                        for k in range(KT):
                            nc.tensor.matmul(
                                ps[:, o:o + n],
                                x1_t[k][:, t * 128:(t + 1) * 128],
                                Wvg_t[k][:, o:o + n],
                                start=(k == 0), stop=(k == KT - 1))
                    vb = pool.tile([128, D], dt.bfloat16, tag=f"vg{t}",
                                   name=f"vg{t}")
                    nc.vector.tensor_copy(vb[:], ps[:])
                    vg_bf.append(vb)
                gs_sb = pool.tile([65, H], dt.float32, tag="gs", name="gs")
                for pt in range(KT):
                    hA, hB = 2 * pt, 2 * pt + 1
                    ps_acc = psm.tile([128, 2], dt.float32, tag="facc",
                                      name="facc", bufs=1)
                    ps_sum = psm.tile([1, 2], dt.float32, tag="fsum",
                                      name="fsum", bufs=1)
                    for j in range(4):
                        ps_s = psm.tile([128, 2], dt.float32, tag="fsgf",
                                        name="fsgf", bufs=1)
                        nc.tensor.matmul(
                            ps_s[:],
                            kgT_bf[pt][:, j * 128:(j + 1) * 128], qg_t[pt][:])
                        e = pool.tile([128, 2], dt.float32, tag="fe",
                                      name="fe", bufs=3)
                        nc.scalar.activation(e[:], ps_s[:], AF.Exp)
                        eb = pool.tile([128, 2], dt.bfloat16, tag="feb",
                                       name="feb", bufs=3)
                        nc.vector.tensor_scalar_mul(eb[:], e[:], gm_t[j][:])
                        nc.tensor.matmul(ps_acc[:],
                                         vg_bf[j][:, pt * 128:(pt + 1) * 128],
                                         eb[:], start=(j == 0), stop=(j == 3))
                        nc.tensor.matmul(ps_sum[:], ones_bf[:], eb[:],
                                         start=(j == 0), stop=(j == 3))
                    nc.vector.tensor_copy(gs_sb[0:64, hA:hA + 1],
                                          ps_acc[0:64, 0:1])
                    nc.vector.tensor_copy(gs_sb[0:64, hB:hB + 1],
                                          ps_acc[64:128, 1:2])
                    nc.vector.tensor_copy(gs_sb[64:65, hA:hA + 1],
                                          ps_sum[0:1, 0:1])
                    nc.vector.tensor_copy(gs_sb[64:65, hB:hB + 1],
                                          ps_sum[0:1, 1:2])
                nc.sync.dma_start(gstats_out[:], gs_sb[:])

    nc.compile()
    return nc


# ----------------------------------------------------------------- host math
def _ln_np(x, gamma, beta, eps=1e-5):
    m = x.mean(-1, keepdims=True)
    v = ((x - m) ** 2).mean(-1, keepdims=True)
    return (x - m) / np.sqrt(v + eps) * gamma + beta


def _gelu_tanh(x):
    return 0.5 * x * (1.0 + np.tanh(np.sqrt(2 / np.pi) * (x + 0.044715 * x ** 3)))


def _row_update(x_prev, out0, p):
    a = out0 @ p["Wo"] + p["bo"]
    x = _ln_np(x_prev + a, p["ln1"][0], p["ln1"][1])
    h = _gelu_tanh(x @ p["W1"] + p["b1"])
    return _ln_np(x + h @ p["W2"] + p["b2"], p["ln2"][0], p["ln2"][1])


def _np_params(params):
    out = {}
    for k, v in params.items():
        if isinstance(v, dict):
            out[k] = _np_params(v)
        elif isinstance(v, (list, tuple)):
            out[k] = [_np_params(x) if isinstance(x, dict)
                      else np.asarray(x, np.float32) for x in v]
        else:
            out[k] = np.asarray(v, np.float32)
    return out


def _wtiles(w, ktiles):
    return np.ascontiguousarray(
        np.asarray(w, np.float32).reshape(ktiles, 128, -1)).astype(BF)


def _qg_cols(qg):
    r = qg.reshape(KT, 128)
    out = np.zeros((KT, 128, 2), np.float32)
    out[:, 0:64, 0] = r[:, 0:64]
    out[:, 64:128, 1] = r[:, 64:128]
    return np.ascontiguousarray(out).astype(BF)


def _cols(b, ktiles):
    return np.ascontiguousarray(
        np.asarray(b, np.float32).reshape(ktiles, 128, 1))


def _run_retry(nc, in_maps, **kw):
    import time
    from concourse.bass_utils import run_bass_kernel_spmd
    last = None
    for attempt in range(3):
        try:
            return run_bass_kernel_spmd(nc, in_maps,
                                        core_ids=list(range(NC_CORES)), **kw)
        except Exception as e:  # transient NRT_EXEC_UNIT_UNRECOVERABLE etc.
            last = e
            time.sleep(2.0)
    raise last


def kernel(input_ids, attention_mask, params):

    ids = np.asarray(input_ids).astype(np.int64)[0]
    amask = np.asarray(attention_mask).astype(np.int32)[0]
    P = _np_params(params)
    p1, p2 = P["layers"][0], P["layers"][1]
    tok_emb, pos_emb = P["tok_emb"], P["pos_emb"]

    if "nc1" not in _cache:
        _cache["nc1"] = _build_launch1()
        _cache["nc2"] = _build_launch2()
    nc1, nc2 = _cache["nc1"], _cache["nc2"]

    # ---- per-core launch-1 inputs
    shared = {
        "g_emb": _cols(P["emb_ln"][0], KT), "b_emb": _cols(P["emb_ln"][1], KT),
        "Wq": _wtiles(p1["Wq"] * SCALE, KT), "bq": _cols(p1["bq"] * SCALE, KT),
        "Wk": _wtiles(p1["Wk"], KT), "Wv": _wtiles(p1["Wv"], KT),
        "Wkg": _wtiles(p1["Wkg"], KT), "Wvg": _wtiles(p1["Wvg"], KT),
        "Wqg": _wtiles(p1["Wqg"] * SCALE, KT),
        "bqg": _cols(p1["bqg"] * SCALE, KT),
        "Wo": _wtiles(p1["Wo"], KT),
        "bo": _cols(p1["bo"] + p1["bv"] @ p1["Wo"], KT),
        "g1": _cols(p1["ln1"][0], KT), "be1": _cols(p1["ln1"][1], KT),
        "W1": _wtiles(p1["W1"], KT), "b1": _cols(p1["b1"], MT_FF),
        "W2": _wtiles(p1["W2"], MT_FF), "b2": _cols(p1["b2"], KT),
        "g2": _cols(p1["ln2"][0], KT), "be2": _cols(p1["ln2"][1], KT),
    }
    in_maps = []
    for c in range(NC_CORES):
        start = c * OWN
        gpos = np.arange(start - C, start + OWN + C)
        gposc = np.clip(gpos, 0, S - 1)
        ok = ((gpos >= 0) & (gpos < S)).astype(np.float32)[:, None]
        emb = tok_emb[ids[gposc]] * ok
        pos = pos_emb[gposc] * ok
        embT = (np.concatenate([emb, tok_emb[ids[0]][None]], 0)
                + np.concatenate([pos, pos_emb[0][None]], 0)).T
        maskT = np.zeros((2, 768, 256), np.float32)
        for lt in range(2):
            t = 2 * c + lt
            j = np.arange(768)[:, None]; qi = np.arange(256)[None, :]
            kp = (t - 1) * C + j
            valid = (np.abs(j - C - qi) <= C) & (kp >= 0) & (kp < S) & (kp != 0)
            pad_ok = amask[np.clip(kp, 0, S - 1)] > 0
            maskT[lt] = (valid & pad_ok).astype(np.float32)
        m = dict(shared)
        m["embT"] = np.ascontiguousarray(embT.reshape(KT, 128, EXT + 1))
        m["maskT"] = np.ascontiguousarray(
            maskT.reshape(2, 6, 128, 256)).astype(BF)
        m["gmask"] = np.ascontiguousarray(
            amask[start:start + OWN].astype(np.float32).reshape(4, 128, 1))
        in_maps.append(m)

    _cache["in_maps_nc1"] = in_maps
    res1 = _run_retry(nc1, in_maps)

    # ---- host: reduce layer-1 global stats -> x1[0]
    accs = np.zeros((H, HD), np.float64)
    sums = np.zeros(H, np.float64)
    for c in range(NC_CORES):
        gs = res1.results[c]["gstats"].astype(np.float64)
        accs += gs[:64].T.reshape(H, HD)
        sums += gs[64]
    out0 = (accs / sums[:, None]).astype(np.float32).reshape(-1) + p1["bvg"]
    x0row = res1.results[0]["x0row"].reshape(D)
    x1_0 = _row_update(x0row, out0, p1)

    # ---- launch 2
    x1T_by_core = []
    for c in range(NC_CORES):
        x1c = res1.results[c]["x1T"].reshape(D, OWN).copy()
        if c == 0:
            x1c[:, 0] = x1_0
        x1T_by_core.append(x1c)
    qg2 = x1_0 @ (p2["Wqg"] * SCALE) + p2["bqg"] * SCALE
    shared2 = {
        "Wkg": _wtiles(p2["Wkg"], KT), "Wvg": _wtiles(p2["Wvg"], KT),
        "qgT": _qg_cols(qg2),
    }
    in_maps2 = []
    for c in range(NC_CORES):
        m = dict(shared2)
        m["x1T"] = np.ascontiguousarray(
            x1T_by_core[c].reshape(KT, 128, OWN)).astype(BF)
        m["gmask"] = np.ascontiguousarray(
            amask[c * OWN:(c + 1) * OWN].astype(np.float32).reshape(4, 128, 1))
        in_maps2.append(m)
    _cache["in_maps_nc2"] = in_maps2
    res2 = _run_retry(nc2, in_maps2)

    accs2 = np.zeros((H, HD), np.float64)
    sums2 = np.zeros(H, np.float64)
    for c in range(NC_CORES):
        gs = res2.results[c]["gstats"].astype(np.float64)
        accs2 += gs[:64].T.reshape(H, HD)
        sums2 += gs[64]
    out0_2 = (accs2 / sums2[:, None]).astype(np.float32).reshape(-1) + p2["bvg"]
    x2_0 = _row_update(x1_0, out0_2, p2)
    logits = x2_0 @ P["clf_W"] + P["clf_b"]
    return logits[None, :].astype(np.float32)


# revision 29
# speedup vs baseline: 1.0047x; 1.0047x over previous
"""Trainium2 Bass kernel for nn_LongformerClassifier (sparse_attention).

Strategy (validated against the reference in numpy first):
  - The model output is only the CLS-token logits (1, 50), so layer 2
    collapses to: global attention of token 0 over all x1 + one-row FFN.
  - Launch 1 (8 cores, sequence-parallel, 512 own tokens + 256 halo each
    side): embeddings + LN, full layer-1 (sliding-window + global-column
    attention, FFN), plus per-core partial softmax stats for layer-1
    global attention of token 0.
  - Host: reduce the tiny per-core stats -> x1[0] row (couple of matvecs).
  - Launch 2 (8 cores): per-core partial stats for layer-2 global
    attention of token 0 over x1.
  - Host: reduce -> layer-2 row FFN -> classifier -> logits.

  All activations are kept feature-major ([feature partitions, token
  free dim]) so no on-device transposes are needed anywhere; softmax is
  computed key-major without max-subtraction (scores are O(1) here) and
  partition-dim reductions are done with ones-vector matmuls.
  Matmuls run in bf16 (fp32 PSUM accumulation).
"""
import sys
import numpy as np
import ml_dtypes

sys.path.insert(0, "/opt/trn_rl_repo")

V, S, D, H, NUM_LABELS = 50265, 4096, 768, 12, 50
HD, C, FF = 64, 256, 3072
NC_CORES = 8
OWN = S // NC_CORES        # 512
EXT = OWN + 2 * C          # 1024 (token 0 appended as column EXT)
SCALE = 1.0 / np.sqrt(HD)
KT = D // 128              # 6 feature partition-tiles
MT_FF = FF // 128          # 24
BF = ml_dtypes.bfloat16

_cache = {}


# ----------------------------------------------------------------- bass build
def _mk(nc_mod, bacc_mod):
    pass


def _build_launch1():
    import concourse.bacc as bacc
    import concourse.mybir as mybir
    import concourse.tile as tile

    dt = mybir.dt
    AF = mybir.ActivationFunctionType
    ALU = mybir.AluOpType
    nc = bacc.Bacc("TRN2", target_bir_lowering=False, debug=False,
                   num_devices=NC_CORES)

    def din(name, shape, dtype=dt.float32):
        return nc.dram_tensor(name, shape, dtype, kind="ExternalInput").ap()

    def dout(name, shape, dtype=dt.float32):
        return nc.dram_tensor(name, shape, dtype, kind="ExternalOutput").ap()

    NEXT = EXT + 1  # 1025
    embT = din("embT", [KT, 128, NEXT])
    g_emb = din("g_emb", [KT, 128, 1]); b_emb = din("b_emb", [KT, 128, 1])
    Wq = din("Wq", [KT, 128, D], dt.bfloat16); bq = din("bq", [KT, 128, 1])
    Wk = din("Wk", [KT, 128, D], dt.bfloat16)
    Wv = din("Wv", [KT, 128, D], dt.bfloat16)
    Wkg = din("Wkg", [KT, 128, D], dt.bfloat16)
    Wvg = din("Wvg", [KT, 128, D], dt.bfloat16)
    Wqg = din("Wqg", [KT, 128, D], dt.bfloat16); bqg = din("bqg", [KT, 128, 1])
    Wo = din("Wo", [KT, 128, D], dt.bfloat16); bo = din("bo", [KT, 128, 1])
    g1 = din("g1", [KT, 128, 1]); be1 = din("be1", [KT, 128, 1])
    W1 = din("W1", [KT, 128, FF], dt.bfloat16); b1 = din("b1", [MT_FF, 128, 1])
    W2 = din("W2", [MT_FF, 128, D], dt.bfloat16); b2 = din("b2", [KT, 128, 1])
    g2 = din("g2", [KT, 128, 1]); be2 = din("be2", [KT, 128, 1])
    maskT = din("maskT", [2, 6, 128, 256], dt.bfloat16)
    gmask = din("gmask", [4, 128, 1])

    x1T_out = dout("x1T", [KT, 128, OWN])
    gstats_out = dout("gstats", [65, H])
    x0row_out = dout("x0row", [KT, 128, 1])

    with tile.TileContext(nc) as tc:
        import contextlib
        with contextlib.ExitStack() as ctx:
            const = ctx.enter_context(tc.tile_pool(name="const", bufs=1))
            ones_bf = const.tile([128, 1], dt.bfloat16, tag="ones_bf",
                                 name="ones_bf")
            nc.vector.memset(ones_bf[:], 1.0)
            eps_t = const.tile([1, 1], dt.float32, tag="eps_t", name="eps_t")
            nc.vector.memset(eps_t[:], 1e-5)

            perm = ctx.enter_context(tc.tile_pool(name="perm", bufs=1))
            trans = ctx.enter_context(tc.tile_pool(name="trans", bufs=2))
            wpool = ctx.enter_context(tc.tile_pool(name="wpool", bufs=1))

            def load_cols(ap, n, dtype=dt.float32):
                ts = []
                for i in range(n):
                    t = perm.tile([128, 1], dtype,
                                  tag=f"pp{ap.tensor.name}{i}",
                                  name=f"pp{ap.tensor.name}{i}")
                    nc.sync.dma_start(t[:], ap[i])
                    ts.append(t)
                return ts

            bq_t = load_cols(bq, KT); bqg_t = load_cols(bqg, KT)
            bo_t = load_cols(bo, KT); b1_t = load_cols(b1, MT_FF)
            b2_t = load_cols(b2, KT)
            g_emb_t = load_cols(g_emb, KT); b_emb_t = load_cols(b_emb, KT)
            g1_t = load_cols(g1, KT); be1_t = load_cols(be1, KT)
            g2_t = load_cols(g2, KT); be2_t = load_cols(be2, KT)
            gmask_t = load_cols(gmask, 4)

            # weight slots: small double-buffered for 768-wide matrices,
            # big single-buffered only for W1
            def load_w_shared(ap, ntiles, cols, big=False):
                ts = []
                for k in range(ntiles):
                    if big:
                        t = wpool.tile([128, cols], dt.bfloat16, tag=f"wld{k}",
                                       name=f"wld{k}", bufs=1)
                    else:
                        t = wpool.tile([128, cols], dt.bfloat16, tag=f"wsm{k}",
                                       name=f"wsm{k}", bufs=2)
                    nc.sync.dma_start(t[:], ap[k])
                    ts.append(t)
                return ts

            # ------------- feature-major layernorm (in-place capable) -------
            def ln_feat(xf32, ncols, gam, bet, out_tags, want_bf, bf_tags):
                nspl = [(o, min(512, ncols - o)) for o in range(0, ncols, 512)]
                pre_bf = []
                for k in range(KT):
                    pb = perm.tile([128, ncols], dt.bfloat16, tag=f"preb{k}",
                                   name=f"preb{k}")
                    nc.vector.tensor_copy(pb[:], xf32[k][:])
                    pre_bf.append(pb)
                srow = trans.tile([1, NEXT], dt.float32, tag="ln_srow",
                                  name="ln_srow", bufs=1)
                qrow = trans.tile([1, NEXT], dt.float32, tag="ln_qrow",
                                  name="ln_qrow", bufs=1)
                with tc.tile_pool(name="lnps", bufs=2, space="PSUM") as psm:
                    for (o, n) in nspl:
                        ps = psm.tile([1, 512], dt.float32, tag="ln_ps",
                                      name="ln_ps")
                        ps2 = psm.tile([1, 512], dt.float32, tag="ln_ps2",
                                       name="ln_ps2")
                        for k in range(KT):
                            nc.tensor.matmul(ps[:, :n], ones_bf[:],
                                             pre_bf[k][:, o:o + n],
                                             start=(k == 0),
                                             stop=(k == KT - 1))
                            sqc = trans.tile([128, 512], dt.bfloat16,
                                             tag="lnsqc", name="lnsqc",
                                             bufs=2)
                            nc.vector.tensor_mul(sqc[:, :n],
                                                 pre_bf[k][:, o:o + n],
                                                 pre_bf[k][:, o:o + n])
                            nc.tensor.matmul(ps2[:, :n], ones_bf[:],
                                             sqc[:, :n],
                                             start=(k == 0),
                                             stop=(k == KT - 1))
                        nc.vector.tensor_scalar_mul(srow[:, o:o + n],
                                                    ps[:, :n], -1.0 / D)
                        nc.vector.tensor_scalar_mul(qrow[:, o:o + n],
                                                    ps2[:, :n], 1.0 / D)
                m2 = trans.tile([1, NEXT], dt.float32, tag="ln_m2",
                                name="ln_m2", bufs=1)
                rst = trans.tile([1, NEXT], dt.float32, tag="ln_rst",
                                name="ln_rst", bufs=1)
                nc.vector.tensor_mul(m2[:, :ncols], srow[:, :ncols],
                                     srow[:, :ncols])
                nc.vector.tensor_sub(qrow[:, :ncols], qrow[:, :ncols],
                                     m2[:, :ncols])
                nc.scalar.activation(m2[:, :ncols], qrow[:, :ncols],
                                     AF.Sqrt, bias=eps_t[:])
                nc.vector.reciprocal(rst[:, :ncols], m2[:, :ncols])
                nm_b = trans.tile([128, NEXT], dt.float32, tag="ln_nmb",
                                  name="ln_nmb", bufs=1)
                rs_b = trans.tile([128, NEXT], dt.float32, tag="ln_rsb",
                                  name="ln_rsb", bufs=1)
                nc.gpsimd.partition_broadcast(nm_b[:, :ncols], srow[:, :ncols])
                nc.gpsimd.partition_broadcast(rs_b[:, :ncols], rst[:, :ncols])
                out_f32, out_bf = [], []
                for k in range(KT):
                    xc = trans.tile([128, NEXT], dt.float32, tag="ln_xc",
                                    name="ln_xc", bufs=1)
                    nc.vector.tensor_add(xc[:, :ncols], xf32[k][:],
                                         nm_b[:, :ncols])
                    nc.vector.tensor_mul(xc[:, :ncols], xc[:, :ncols],
                                         rs_b[:, :ncols])
                    of = perm.tile([128, ncols], dt.float32, tag=out_tags[k],
                                   name=out_tags[k])
                    nc.vector.tensor_scalar(of[:], xc[:, :ncols], gam[k][:],
                                            bet[k][:], ALU.mult, ALU.add)
                    out_f32.append(of)
                    if want_bf:
                        ob = perm.tile([128, ncols], dt.bfloat16,
                                       tag=bf_tags[k], name=bf_tags[k])
                        nc.vector.tensor_copy(ob[:], of[:])
                        out_bf.append(ob)
                return out_f32, out_bf

            # ---------------- phase A: embeddings + emb LN -------------------
            xpre = []
            for k in range(KT):
                xz = perm.tile([128, NEXT], dt.float32, tag=f"xz{k}",
                               name=f"xz{k}")
                nc.sync.dma_start(xz[:, 0:513], embT[k][:, 0:513])
                nc.sync.dma_start(xz[:, 513:NEXT], embT[k][:, 513:NEXT])
                xpre.append(xz)
            x0T, x0T_bf = ln_feat(xpre, NEXT, g_emb_t, b_emb_t,
                                  [f"xz{k}" for k in range(KT)],
                                  True, [f"preb{k}" for k in range(KT)])
            for k in range(KT):
                nc.sync.dma_start(x0row_out[k], x0T[k][:, EXT:EXT + 1])

            # ---------------- phase B: projections ---------------------------
            def proj_fm(col0, ncols, bias_ts, tag, psm, wtiles):
                outs = []
                nspl = [(o, min(512, ncols - o)) for o in range(0, ncols, 512)]
                for m in range(KT):
                    out = perm.tile([128, ncols], dt.bfloat16, tag=f"{tag}{m}",
                                    name=f"{tag}{m}")
                    for (o, n) in nspl:
                        ps = psm.tile([128, 512], dt.float32, tag="projps",
                                      name="projps")
                        for k in range(KT):
                            nc.tensor.matmul(
                                ps[:, :n],
                                wtiles[k][:, m * 128:(m + 1) * 128],
                                x0T_bf[k][:, col0 + o:col0 + o + n],
                                start=(k == 0), stop=(k == KT - 1))
                        if bias_ts is None:
                            nc.vector.tensor_copy(out[:, o:o + n], ps[:, :n])
                        else:
                            nc.scalar.activation(out[:, o:o + n], ps[:, :n],
                                                 AF.Identity,
                                                 bias=bias_ts[m][:])
                    outs.append(out)
                return outs

            with tc.tile_pool(name="psB", bufs=4, space="PSUM") as psB:
                Wk_t = load_w_shared(Wk, KT, D)
                KT_bf = proj_fm(0, NEXT, None, "ad", psB, Wk_t)
                Wq_t = load_w_shared(Wq, KT, D)
                QT_bf = proj_fm(C, OWN, bq_t, "QTb", psB, Wq_t)
                Wkg_t = load_w_shared(Wkg, KT, D)
                kgT_bf = proj_fm(C, OWN, None, "kgTb", psB, Wkg_t)
                Wqg_t = load_w_shared(Wqg, KT, D)
                qgT_bf = proj_fm(EXT, 1, bqg_t, "qgTb", psB, Wqg_t)

            with tc.tile_pool(name="psV", bufs=1, space="PSUM") as psV:
                Wv_t = load_w_shared(Wv, KT, D)
                V_bf = []
                for t in range(8):
                    ps = psV.tile([128, D], dt.float32, tag="vps",
                                  name="vps", bufs=2)
                    for (o, n) in [(0, 512), (512, 256)]:
                        for k in range(KT):
                            nc.tensor.matmul(
                                ps[:, o:o + n],
                                x0T_bf[k][:, t * 128:(t + 1) * 128],
                                Wv_t[k][:, o:o + n],
                                start=(k == 0), stop=(k == KT - 1))
                    vb = perm.tile([128, H, 65], dt.bfloat16, tag=f"Vb{t}",
                                   name=f"Vb{t}")
                    for h in range(H):
                        nc.vector.tensor_copy(vb[:, h, 0:64],
                                              ps[:, h * 64:(h + 1) * 64])
                    nc.vector.memset(vb[:, :, 64:65], 1.0)
                    V_bf.append(vb)
                v0_bf = perm.tile([1, H, 65], dt.bfloat16, tag="v0b",
                                  name="v0b")
                ps0 = psV.tile([1, D], dt.float32, tag="v0ps", name="v0ps",
                               bufs=1)
                for (o, n) in [(0, 512), (512, 256)]:
                    for k in range(KT):
                        nc.tensor.matmul(ps0[:, o:o + n],
                                         x0T_bf[k][:, EXT:EXT + 1],
                                         Wv_t[k][:, o:o + n],
                                         start=(k == 0), stop=(k == KT - 1))
                for h in range(H):
                    nc.vector.tensor_copy(v0_bf[:, h, 0:64],
                                          ps0[:, h * 64:(h + 1) * 64])
                nc.vector.memset(v0_bf[:, :, 64:65], 1.0)
                Wvg_t = load_w_shared(Wvg, KT, D)
                vg_bf = []
                for t in range(4):
                    ps = psV.tile([128, D], dt.float32, tag="vps",
                                  name="vps", bufs=2)
                    for (o, n) in [(0, 512), (512, 256)]:
                        for k in range(KT):
                            nc.tensor.matmul(
                                ps[:, o:o + n],
                                x0T_bf[k][:, C + t * 128:C + (t + 1) * 128],
                                Wvg_t[k][:, o:o + n],
                                start=(k == 0), stop=(k == KT - 1))
                    vb = perm.tile([128, D], dt.bfloat16, tag=f"vgb{t}",
                                   name=f"vgb{t}")
                    nc.vector.tensor_copy(vb[:], ps[:])
                    vg_bf.append(vb)

            mk_t = [[None] * 6 for _ in range(2)]
            for lt in range(2):
                for i in range(6):
                    mt_ = perm.tile([128, 256], dt.bfloat16,
                                    tag=f"mk{lt}_{i}", name=f"mk{lt}_{i}")
                    nc.sync.dma_start(mt_[:], maskT[lt, i])
                    mk_t[lt][i] = mt_

            # ---------------- phase F: layer-1 global stats ------------------
            gs_sb = perm.tile([65, H], dt.float32, tag="gs", name="gs")
            with tc.tile_pool(name="psF", bufs=2, space="PSUM") as psF:
                for pt in range(KT):
                    hA, hB = 2 * pt, 2 * pt + 1
                    qg2c = trans.tile([128, 2], dt.bfloat16, tag="qg2c",
                                      name="qg2c", bufs=2)
                    nc.vector.memset(qg2c[:], 0.0)
                    nc.vector.tensor_copy(qg2c[0:64, 0:1], qgT_bf[pt][0:64, :])
                    nc.vector.tensor_copy(qg2c[64:128, 1:2],
                                          qgT_bf[pt][64:128, :])
                    ps_acc = psF.tile([128, 2], dt.float32, tag="facc",
                                      name="facc")
                    ps_sum = psF.tile([1, 2], dt.float32, tag="fsum",
                                      name="fsum")
                    for j in range(4):
                        ps_s = psF.tile([128, 2], dt.float32, tag="fsgf",
                                        name="fsgf")
                        nc.tensor.matmul(
                            ps_s[:],
                            kgT_bf[pt][:, j * 128:(j + 1) * 128], qg2c[:])
                        e = trans.tile([128, 2], dt.float32, tag="fe",
                                       name="fe")
                        nc.scalar.activation(e[:], ps_s[:], AF.Exp)
                        eb = trans.tile([128, 2], dt.bfloat16, tag="feb",
                                        name="feb")
                        nc.vector.tensor_scalar_mul(eb[:], e[:],
                                                    gmask_t[j][:])
                        nc.tensor.matmul(ps_acc[:],
                                         vg_bf[j][:, pt * 128:(pt + 1) * 128],
                                         eb[:], start=(j == 0), stop=(j == 3))
                        nc.tensor.matmul(ps_sum[:], ones_bf[:], eb[:],
                                         start=(j == 0), stop=(j == 3))
                    nc.vector.tensor_copy(gs_sb[0:64, hA:hA + 1],
                                          ps_acc[0:64, 0:1])
                    nc.vector.tensor_copy(gs_sb[0:64, hB:hB + 1],
                                          ps_acc[64:128, 1:2])
                    nc.vector.tensor_copy(gs_sb[64:65, hA:hA + 1],
                                          ps_sum[0:1, 0:1])
                    nc.vector.tensor_copy(gs_sb[64:65, hB:hB + 1],
                                          ps_sum[0:1, 1:2])
            nc.sync.dma_start(gstats_out[:], gs_sb[:])

            # ---------------- phase C: windowed attention --------------------
            attnT_bf = []
            for k in range(KT):
                at = perm.tile([128, OWN], dt.bfloat16, tag=f"at{k}",
                               name=f"at{k}")
                attnT_bf.append(at)
            with (tc.tile_pool(name="psS", bufs=1, space="PSUM") as psS,
                  tc.tile_pool(name="psG", bufs=1, space="PSUM") as psG,
                  tc.tile_pool(name="psO", bufs=2, space="PSUM") as psO):
                def stage1(h, lt):
                    pt, ro = h // 2, (h % 2) * 64
                    qs = QT_bf[pt][ro:ro + 64, lt * 256:(lt + 1) * 256]
                    es = trans.tile([128, 6, 256], dt.bfloat16, tag="es",
                                    name="es", bufs=2)
                    for w in range(2):
                        ps_s = psS.tile([128, 3, 256], dt.float32,
                                        tag="ps_s", name="ps_s", bufs=2)
                        for i3 in range(3):
                            i = w * 3 + i3
                            nc.tensor.matmul(
                                ps_s[:, i3, :],
                                KT_bf[pt][ro:ro + 64,
                                          lt * 256 + i * 128:
                                          lt * 256 + (i + 1) * 128],
                                qs)
                            nc.scalar.activation(es[:, i, :],
                                                 ps_s[:, i3, :], AF.Exp)
                            nc.vector.tensor_mul(es[:, i, :], es[:, i, :],
                                                 mk_t[lt][i][:])
                    ps_sg = psG.tile([1, 256], dt.float32, tag="ps_sg",
                                     name="ps_sg")
                    nc.tensor.matmul(ps_sg[:],
                                     KT_bf[pt][ro:ro + 64, EXT:EXT + 1], qs)
                    esg = trans.tile([1, 256], dt.bfloat16, tag="esg",
                                     name="esg", bufs=2)
                    nc.scalar.activation(esg[:], ps_sg[:], AF.Exp)
                    return es, esg

                def stage2(h, lt, es, esg):
                    pt, ro = h // 2, (h % 2) * 64
                    ps_o = psO.tile([65, 256], dt.float32, tag="ps_o",
                                    name="ps_o")
                    for i in range(6):
                        nc.tensor.matmul(
                            ps_o[:],
                            V_bf[lt * 2 + i][:, h, :],
                            es[:, i, :], start=(i == 0), stop=False)
                    nc.tensor.matmul(ps_o[:], v0_bf[:, h, :],
                                     esg[:], start=False, stop=True)
                    rrow = trans.tile([1, 256], dt.float32, tag="rrow",
                                      name="rrow", bufs=2)
                    nc.vector.reciprocal(rrow[:], ps_o[64:65, :])
                    rb = trans.tile([64, 256], dt.float32, tag="rb",
                                    name="rb", bufs=2)
                    nc.gpsimd.partition_broadcast(rb[:], rrow[:])
                    nc.vector.tensor_mul(
                        attnT_bf[pt][ro:ro + 64, lt * 256:(lt + 1) * 256],
                        ps_o[0:64, :], rb[:])

                pend = None
                for pt in range(KT):
                    for lt in range(2):
                        for h in (2 * pt, 2 * pt + 1):
                            cur = stage1(h, lt)
                            if pend is not None:
                                stage2(*pend)
                            pend = (h, lt, *cur)
                stage2(*pend)

            # ---------------- phase D: attn proj + residual + LN1 ------------
            with tc.tile_pool(name="psD", bufs=4, space="PSUM") as psD:
                Wo_t = load_w_shared(Wo, KT, D)
                apre = []
                for m in range(KT):
                    ps = psD.tile([128, OWN], dt.float32, tag="dps",
                                  name="dps")
                    for k in range(KT):
                        nc.tensor.matmul(ps[:],
                                         Wo_t[k][:, m * 128:(m + 1) * 128],
                                         attnT_bf[k][:],
                                         start=(k == 0), stop=(k == KT - 1))
                    asb = trans.tile([128, OWN], dt.float32, tag="asb",
                                     name="asb")
                    nc.scalar.activation(asb[:], ps[:], AF.Identity,
                                         bias=bo_t[m][:])
                    ad = perm.tile([128, OWN], dt.float32, tag=f"ad{m}",
                                   name=f"ad{m}")
                    nc.vector.tensor_add(ad[:], asb[:], x0T[m][:, C:C + OWN])
                    apre.append(ad)
            xmT, xmT_bf = ln_feat(apre, OWN, g1_t, be1_t,
                                  [f"ad{k}" for k in range(KT)],
                                  True, [f"Vb{k}" for k in range(KT)])

            # ---------------- phase E: FFN + residual + LN2 ------------------
            with tc.tile_pool(name="psE", bufs=1, space="PSUM") as psE:
                W1_t = load_w_shared(W1, KT, FF, big=True)
                yps = [psE.tile([128, OWN], dt.float32, tag=f"yps{m}",
                                name=f"yps{m}") for m in range(KT)]
                ypre = []
                for m in range(MT_FF):
                    ps = psE.tile([128, OWN], dt.float32, tag="w1ps",
                                  name="w1ps", bufs=2)
                    for k in range(KT):
                        nc.tensor.matmul(ps[:],
                                         W1_t[k][:, m * 128:(m + 1) * 128],
                                         xmT_bf[k][:],
                                         start=(k == 0), stop=(k == KT - 1))
                    ht = trans.tile([128, OWN], dt.bfloat16, tag="hT",
                                    name="hT", bufs=3)
                    nc.scalar.activation(ht[:], ps[:], AF.Gelu_apprx_tanh,
                                         bias=b1_t[m][:])
                    w2t = trans.tile([128, D], dt.bfloat16, tag="w2ld",
                                     name="w2ld", bufs=3)
                    nc.sync.dma_start(w2t[:], W2[m])
                    for mo in range(KT):
                        nc.tensor.matmul(yps[mo][:],
                                         w2t[:, mo * 128:(mo + 1) * 128],
                                         ht[:], start=(m == 0),
                                         stop=(m == MT_FF - 1))
                for m in range(KT):
                    ysb = trans.tile([128, OWN], dt.float32, tag="ysb",
                                     name="ysb")
                    nc.scalar.activation(ysb[:], yps[m][:], AF.Identity,
                                         bias=b2_t[m][:])
                    yz = perm.tile([128, OWN], dt.float32, tag=f"xz{m}",
                                   name=f"xz{m}")
                    nc.vector.tensor_add(yz[:], ysb[:], xmT[m][:])
                    ypre.append(yz)
            x1T, _ = ln_feat(ypre, OWN, g2_t, be2_t,
                             [f"xz{k}" for k in range(KT)], False, None)
            for k in range(KT):
                nc.sync.dma_start(x1T_out[k], x1T[k][:])

    nc.compile()
    return nc


def _build_launch2():
    import concourse.bacc as bacc
    import concourse.mybir as mybir
    import concourse.tile as tile

    dt = mybir.dt
    AF = mybir.ActivationFunctionType
    nc = bacc.Bacc("TRN2", target_bir_lowering=False, debug=False,
                   num_devices=NC_CORES)

    def din(name, shape, dtype=dt.float32):
        return nc.dram_tensor(name, shape, dtype, kind="ExternalInput").ap()

    x1T = din("x1T", [KT, 128, OWN], dt.bfloat16)
    Wkg = din("Wkg", [KT, 128, D], dt.bfloat16)
    Wvg = din("Wvg", [KT, 128, D], dt.bfloat16)
    qgT = din("qgT", [KT, 128, 2], dt.bfloat16)   # zero-padded head-pair cols
    gmask = din("gmask", [4, 128, 1])
    gstats_out = nc.dram_tensor("gstats", [65, H], dt.float32,
                                kind="ExternalOutput").ap()

    with tile.TileContext(nc) as tc:
        import contextlib
        with contextlib.ExitStack() as ctx:
            pool = ctx.enter_context(tc.tile_pool(name="pool", bufs=1))
            const = ctx.enter_context(tc.tile_pool(name="const", bufs=1))
            ones_bf = const.tile([128, 1], dt.bfloat16, tag="ones_bf",
                                 name="ones_bf")
            nc.vector.memset(ones_bf[:], 1.0)
            x1_t, qg_t, Wkg_t, Wvg_t = [], [], [], []
            for k in range(KT):
                t = pool.tile([128, OWN], dt.bfloat16, tag=f"x1{k}",
                              name=f"x1{k}")
                nc.sync.dma_start(t[:], x1T[k])
                x1_t.append(t)
                t = pool.tile([128, D], dt.bfloat16, tag=f"Wkg{k}",
                              name=f"Wkg{k}")
                nc.sync.dma_start(t[:], Wkg[k])
                Wkg_t.append(t)
            for k in range(KT):
                t = pool.tile([128, 2], dt.bfloat16, tag=f"qg{k}",
                              name=f"qg{k}")
                nc.sync.dma_start(t[:], qgT[k])
                qg_t.append(t)
                t = pool.tile([128, D], dt.bfloat16, tag=f"Wvg{k}",
                              name=f"Wvg{k}")
                nc.sync.dma_start(t[:], Wvg[k])
                Wvg_t.append(t)
            gm_t = []
            for j in range(4):
                t = pool.tile([128, 1], dt.float32, tag=f"gm{j}",
                              name=f"gm{j}")
                nc.sync.dma_start(t[:], gmask[j])
                gm_t.append(t)

            with tc.tile_pool(name="ps", bufs=1, space="PSUM") as psm:
                kgT_bf = []
                for m in range(KT):
                    ps = psm.tile([128, OWN], dt.float32, tag="kps",
                                  name="kps", bufs=2)
                    for k in range(KT):
                        nc.tensor.matmul(ps[:],
                                         Wkg_t[k][:, m * 128:(m + 1) * 128],
                                         x1_t[k][:],
                                         start=(k == 0), stop=(k == KT - 1))
                    kg = pool.tile([128, OWN], dt.bfloat16, tag=f"kg{m}",
                                   name=f"kg{m}")
                    nc.vector.tensor_copy(kg[:], ps[:])
                    kgT_bf.append(kg)
                vg_bf = []
                for t in range(4):
                    ps = psm.tile([128, D], dt.float32, tag="vps",
                                  name="vps", bufs=1)
                    for (o, n) in [(0, 512), (512, 256)]:
                        for k in range(KT):
                            nc.tensor.matmul(
                                ps[:, o:o + n],
                                x1_t[k][:, t * 128:(t + 1) * 128],
                                Wvg_t[k][:, o:o + n],
                                start=(k == 0), stop=(k == KT - 1))
                    vb = pool.tile([128, D], dt.bfloat16, tag=f"vg{t}",
                                   name=f"vg{t}")
                    nc.vector.tensor_copy(vb[:], ps[:])
                    vg_bf.append(vb)
                gs_sb = pool.tile([65, H], dt.float32, tag="gs", name="gs")
                for pt in range(KT):
                    hA, hB = 2 * pt, 2 * pt + 1
                    ps_acc = psm.tile([128, 2], dt.float32, tag="facc",
                                      name="facc", bufs=1)
                    ps_sum = psm.tile([1, 2], dt.float32, tag="fsum",
                                      name="fsum", bufs=1)
                    for j in range(4):
                        ps_s = psm.tile([128, 2], dt.float32, tag="fsgf",
                                        name="fsgf", bufs=1)
                        nc.tensor.matmul(
                            ps_s[:],
                            kgT_bf[pt][:, j * 128:(j + 1) * 128], qg_t[pt][:])
                        e = pool.tile([128, 2], dt.float32, tag="fe",
                                      name="fe", bufs=3)
                        nc.scalar.activation(e[:], ps_s[:], AF.Exp)
                        eb = pool.tile([128, 2], dt.bfloat16, tag="feb",
                                       name="feb", bufs=3)
                        nc.vector.tensor_scalar_mul(eb[:], e[:], gm_t[j][:])
                        nc.tensor.matmul(ps_acc[:],
                                         vg_bf[j][:, pt * 128:(pt + 1) * 128],
                                         eb[:], start=(j == 0), stop=(j == 3))
                        nc.tensor.matmul(ps_sum[:], ones_bf[:], eb[:],
                                         start=(j == 0), stop=(j == 3))
                    nc.vector.tensor_copy(gs_sb[0:64, hA:hA + 1],
                                          ps_acc[0:64, 0:1])
                    nc.vector.tensor_copy(gs_sb[0:64, hB:hB + 1],
                                          ps_acc[64:128, 1:2])
                    nc.vector.tensor_copy(gs_sb[64:65, hA:hA + 1],
                                          ps_sum[0:1, 0:1])
                    nc.vector.tensor_copy(gs_sb[64:65, hB:hB + 1],
                                          ps_sum[0:1, 1:2])
                nc.sync.dma_start(gstats_out[:], gs_sb[:])

    nc.compile()
    return nc


# ----------------------------------------------------------------- host math
def _ln_np(x, gamma, beta, eps=1e-5):
    m = x.mean(-1, keepdims=True)
    v = ((x - m) ** 2).mean(-1, keepdims=True)
    return (x - m) / np.sqrt(v + eps) * gamma + beta


def _gelu_tanh(x):
    return 0.5 * x * (1.0 + np.tanh(np.sqrt(2 / np.pi) * (x + 0.044715 * x ** 3)))


def _row_update(x_prev, out0, p):
    a = out0 @ p["Wo"] + p["bo"]
    x = _ln_np(x_prev + a, p["ln1"][0], p["ln1"][1])
    h = _gelu_tanh(x @ p["W1"] + p["b1"])
    return _ln_np(x + h @ p["W2"] + p["b2"], p["ln2"][0], p["ln2"][1])


def _np_params(params):
    out = {}
    for k, v in params.items():
        if isinstance(v, dict):
            out[k] = _np_params(v)
        elif isinstance(v, (list, tuple)):
            out[k] = [_np_params(x) if isinstance(x, dict)
                      else np.asarray(x, np.float32) for x in v]
        else:
            out[k] = np.asarray(v, np.float32)
    return out


def _wtiles(w, ktiles):
    return np.ascontiguousarray(
        np.asarray(w, np.float32).reshape(ktiles, 128, -1)).astype(BF)


def _qg_cols(qg):
    r = qg.reshape(KT, 128)
    out = np.zeros((KT, 128, 2), np.float32)
    out[:, 0:64, 0] = r[:, 0:64]
    out[:, 64:128, 1] = r[:, 64:128]
    return np.ascontiguousarray(out).astype(BF)


def _cols(b, ktiles):
    return np.ascontiguousarray(
        np.asarray(b, np.float32).reshape(ktiles, 128, 1))


def _run_retry(nc, in_maps, **kw):
    import time
    from concourse.bass_utils import run_bass_kernel_spmd
    last = None
    for attempt in range(3):
        try:
            return run_bass_kernel_spmd(nc, in_maps,
                                        core_ids=list(range(NC_CORES)), **kw)
        except Exception as e:  # transient NRT_EXEC_UNIT_UNRECOVERABLE etc.
            last = e
            time.sleep(2.0)
    raise last


def kernel(input_ids, attention_mask, params):

    ids = np.asarray(input_ids).astype(np.int64)[0]
    amask = np.asarray(attention_mask).astype(np.int32)[0]
    P = _np_params(params)
    p1, p2 = P["layers"][0], P["layers"][1]
    tok_emb, pos_emb = P["tok_emb"], P["pos_emb"]

    if "nc1" not in _cache:
        _cache["nc1"] = _build_launch1()
        _cache["nc2"] = _build_launch2()
    nc1, nc2 = _cache["nc1"], _cache["nc2"]

    # ---- per-core launch-1 inputs
    shared = {
        "g_emb": _cols(P["emb_ln"][0], KT), "b_emb": _cols(P["emb_ln"][1], KT),
        "Wq": _wtiles(p1["Wq"] * SCALE, KT), "bq": _cols(p1["bq"] * SCALE, KT),
        "Wk": _wtiles(p1["Wk"], KT), "Wv": _wtiles(p1["Wv"], KT),
        "Wkg": _wtiles(p1["Wkg"], KT), "Wvg": _wtiles(p1["Wvg"], KT),
        "Wqg": _wtiles(p1["Wqg"] * SCALE, KT),
        "bqg": _cols(p1["bqg"] * SCALE, KT),
        "Wo": _wtiles(p1["Wo"], KT),
        "bo": _cols(p1["bo"] + p1["bv"] @ p1["Wo"], KT),
        "g1": _cols(p1["ln1"][0], KT), "be1": _cols(p1["ln1"][1], KT),
        "W1": _wtiles(p1["W1"], KT), "b1": _cols(p1["b1"], MT_FF),
        "W2": _wtiles(p1["W2"], MT_FF), "b2": _cols(p1["b2"], KT),
        "g2": _cols(p1["ln2"][0], KT), "be2": _cols(p1["ln2"][1], KT),
    }
    in_maps = []
    for c in range(NC_CORES):
        start = c * OWN
        gpos = np.arange(start - C, start + OWN + C)
        gposc = np.clip(gpos, 0, S - 1)
        ok = ((gpos >= 0) & (gpos < S)).astype(np.float32)[:, None]
        emb = tok_emb[ids[gposc]] * ok
        pos = pos_emb[gposc] * ok
        embT = (np.concatenate([emb, tok_emb[ids[0]][None]], 0)
                + np.concatenate([pos, pos_emb[0][None]], 0)).T
        maskT = np.zeros((2, 768, 256), np.float32)
        for lt in range(2):
            t = 2 * c + lt
            j = np.arange(768)[:, None]; qi = np.arange(256)[None, :]
            kp = (t - 1) * C + j
            valid = (np.abs(j - C - qi) <= C) & (kp >= 0) & (kp < S) & (kp != 0)
            pad_ok = amask[np.clip(kp, 0, S - 1)] > 0
            maskT[lt] = (valid & pad_ok).astype(np.float32)
        m = dict(shared)
        m["embT"] = np.ascontiguousarray(embT.reshape(KT, 128, EXT + 1))
        m["maskT"] = np.ascontiguousarray(
            maskT.reshape(2, 6, 128, 256)).astype(BF)
        m["gmask"] = np.ascontiguousarray(
            amask[start:start + OWN].astype(np.float32).reshape(4, 128, 1))
        in_maps.append(m)

    _cache["in_maps_nc1"] = in_maps
    res1 = _run_retry(nc1, in_maps)

    # ---- host: reduce layer-1 global stats -> x1[0]
    accs = np.zeros((H, HD), np.float64)
    sums = np.zeros(H, np.float64)
    for c in range(NC_CORES):
        gs = res1.results[c]["gstats"].astype(np.float64)
        accs += gs[:64].T.reshape(H, HD)
        sums += gs[64]
    out0 = (accs / sums[:, None]).astype(np.float32).reshape(-1) + p1["bvg"]
    x0row = res1.results[0]["x0row"].reshape(D)
    x1_0 = _row_update(x0row, out0, p1)

    # ---- launch 2
    x1T_by_core = []
    for c in range(NC_CORES):
        x1c = res1.results[c]["x1T"].reshape(D, OWN).copy()
        if c == 0:
            x1c[:, 0] = x1_0
        x1T_by_core.append(x1c)
    qg2 = x1_0 @ (p2["Wqg"] * SCALE) + p2["bqg"] * SCALE
    shared2 = {
        "Wkg": _wtiles(p2["Wkg"], KT), "Wvg": _wtiles(p2["Wvg"], KT),
        "qgT": _qg_cols(qg2),
    }
    in_maps2 = []
    for c in range(NC_CORES):
        m = dict(shared2)
        m["x1T"] = np.ascontiguousarray(
            x1T_by_core[c].reshape(KT, 128, OWN)).astype(BF)
        m["gmask"] = np.ascontiguousarray(
            amask[c * OWN:(c + 1) * OWN].astype(np.float32).reshape(4, 128, 1))
        in_maps2.append(m)
    _cache["in_maps_nc2"] = in_maps2
    res2 = _run_retry(nc2, in_maps2)

    accs2 = np.zeros((H, HD), np.float64)
    sums2 = np.zeros(H, np.float64)
    for c in range(NC_CORES):
        gs = res2.results[c]["gstats"].astype(np.float64)
        accs2 += gs[:64].T.reshape(H, HD)
        sums2 += gs[64]
    out0_2 = (accs2 / sums2[:, None]).astype(np.float32).reshape(-1) + p2["bvg"]
    x2_0 = _row_update(x1_0, out0_2, p2)
    logits = x2_0 @ P["clf_W"] + P["clf_b"]
    return logits[None, :].astype(np.float32)


# revision 33
# speedup vs baseline: 1.2345x; 1.2288x over previous
"""Trainium2 Bass kernel for nn_LongformerClassifier (sparse_attention).

Strategy (validated against the reference in numpy first):
  - The model output is only the CLS-token logits (1, 50), so layer 2
    collapses to: global attention of token 0 over all x1 + one-row FFN.
  - Launch 1 (8 cores, sequence-parallel, 512 own tokens + 256 halo each
    side): embeddings + LN, full layer-1 (sliding-window + global-column
    attention, FFN), plus per-core partial softmax stats for layer-1
    global attention of token 0.
  - Host: reduce the tiny per-core stats -> x1[0] row (couple of matvecs).
  - Launch 2 (8 cores): per-core partial stats for layer-2 global
    attention of token 0 over x1.
  - Host: reduce -> layer-2 row FFN -> classifier -> logits.

  All activations are kept feature-major ([feature partitions, token
  free dim]) so no on-device transposes are needed anywhere; softmax is
  computed key-major without max-subtraction (scores are O(1) here) and
  partition-dim reductions are done with ones-vector matmuls.
  Matmuls run in bf16 (fp32 PSUM accumulation).
"""
import sys
import numpy as np
import ml_dtypes

sys.path.insert(0, "/opt/trn_rl_repo")

V, S, D, H, NUM_LABELS = 50265, 4096, 768, 12, 50
HD, C, FF = 64, 256, 3072
NC_CORES = 8
OWN = S // NC_CORES        # 512
EXT = OWN + 2 * C          # 1024 (token 0 appended as column EXT)
SCALE = 1.0 / np.sqrt(HD)
KT = D // 128              # 6 feature partition-tiles
MT_FF = FF // 128          # 24
BF = ml_dtypes.bfloat16

_cache = {}


# ----------------------------------------------------------------- bass build
def _mk(nc_mod, bacc_mod):
    pass


def _build_launch1():
    import concourse.bacc as bacc
    import concourse.mybir as mybir
    import concourse.tile as tile

    dt = mybir.dt
    AF = mybir.ActivationFunctionType
    ALU = mybir.AluOpType
    nc = bacc.Bacc("TRN2", target_bir_lowering=False, debug=False,
                   num_devices=NC_CORES)

    def din(name, shape, dtype=dt.float32):
        return nc.dram_tensor(name, shape, dtype, kind="ExternalInput").ap()

    def dout(name, shape, dtype=dt.float32):
        return nc.dram_tensor(name, shape, dtype, kind="ExternalOutput").ap()

    NEXT = EXT + 1  # 1025
    embT = din("embT", [KT, 128, NEXT])
    Wq = din("Wq", [KT, 128, D], dt.bfloat16)
    Wk = din("Wk", [KT, 128, D], dt.bfloat16)
    Wv = din("Wv", [KT, 128, D], dt.bfloat16)
    Wkg = din("Wkg", [KT, 128, D], dt.bfloat16)
    Wvg = din("Wvg", [KT, 128, D], dt.bfloat16)
    Wqg = din("Wqg", [KT, 128, D], dt.bfloat16)
    Wo = din("Wo", [KT, 128, D], dt.bfloat16)
    W1 = din("W1", [KT, 128, FF], dt.bfloat16)
    W2 = din("W2", [MT_FF, 128, D], dt.bfloat16)
    maskT = din("maskT", [2, 6, 128, 256], dt.bfloat16)
    smalls = din("smalls", [128, 88])   # packed per-partition params

    x1T_out = dout("x1T", [KT, 128, OWN])
    gstats_out = dout("gstats", [65, H])
    x0row_out = dout("x0row", [KT, 128, 1])

    with tile.TileContext(nc) as tc:
        import contextlib
        with contextlib.ExitStack() as ctx:
            const = ctx.enter_context(tc.tile_pool(name="const", bufs=1))
            ones_bf = const.tile([128, 1], dt.bfloat16, tag="ones_bf",
                                 name="ones_bf")
            nc.vector.memset(ones_bf[:], 1.0)
            eps_t = const.tile([1, 1], dt.float32, tag="eps_t", name="eps_t")
            nc.vector.memset(eps_t[:], 1e-5)

            perm = ctx.enter_context(tc.tile_pool(name="perm", bufs=1))
            trans = ctx.enter_context(tc.tile_pool(name="trans", bufs=2))
            wpool = ctx.enter_context(tc.tile_pool(name="wpool", bufs=1))

            # embeddings first so their DMAs head the queue
            xpre = []
            for k in range(KT):
                xz = perm.tile([128, NEXT], dt.float32, tag=f"xz{k}",
                               name=f"xz{k}")
                nc.sync.dma_start(xz[:, 0:513], embT[k][:, 0:513])
                nc.sync.dma_start(xz[:, 513:NEXT], embT[k][:, 513:NEXT])
                xpre.append(xz)

            sm_t = perm.tile([128, 88], dt.float32, tag="sm", name="sm")
            nc.sync.dma_start(sm_t[:], smalls[:])
            _off = [0]

            def sm_cols(n):
                o = _off[0]; _off[0] += n
                return [sm_t[:, o + i:o + i + 1] for i in range(n)]

            bq_t = sm_cols(KT); bqg_t = sm_cols(KT)
            bo_t = sm_cols(KT); b1_t = sm_cols(MT_FF)
            b2_t = sm_cols(KT)
            g_emb_t = sm_cols(KT); b_emb_t = sm_cols(KT)
            g1_t = sm_cols(KT); be1_t = sm_cols(KT)
            g2_t = sm_cols(KT); be2_t = sm_cols(KT)
            gmask_t = sm_cols(4)

            # weight slots: small double-buffered for 768-wide matrices,
            # big single-buffered only for W1
            def load_w_shared(ap, ntiles, cols, big=False):
                ts = []
                for k in range(ntiles):
                    if big:
                        t = wpool.tile([128, cols], dt.bfloat16, tag=f"wld{k}",
                                       name=f"wld{k}", bufs=1)
                    else:
                        t = wpool.tile([128, cols], dt.bfloat16, tag=f"wsm{k}",
                                       name=f"wsm{k}", bufs=2)
                    nc.sync.dma_start(t[:], ap[k])
                    ts.append(t)
                return ts

            # ------------- feature-major layernorm (in-place capable) -------
            def ln_feat(xf32, ncols, gam, bet, out_tags, want_bf, bf_tags):
                nspl = [(o, min(512, ncols - o)) for o in range(0, ncols, 512)]
                pre_bf = []
                for k in range(KT):
                    pb = perm.tile([128, ncols], dt.bfloat16, tag=f"preb{k}",
                                   name=f"preb{k}")
                    nc.vector.tensor_copy(pb[:], xf32[k][:])
                    pre_bf.append(pb)
                srow = trans.tile([1, NEXT], dt.float32, tag="ln_srow",
                                  name="ln_srow", bufs=1)
                qrow = trans.tile([1, NEXT], dt.float32, tag="ln_qrow",
                                  name="ln_qrow", bufs=1)
                with tc.tile_pool(name="lnps", bufs=2, space="PSUM") as psm:
                    for (o, n) in nspl:
                        ps = psm.tile([1, 512], dt.float32, tag="ln_ps",
                                      name="ln_ps")
                        ps2 = psm.tile([1, 512], dt.float32, tag="ln_ps2",
                                       name="ln_ps2")
                        for k in range(KT):
                            nc.tensor.matmul(ps[:, :n], ones_bf[:],
                                             pre_bf[k][:, o:o + n],
                                             start=(k == 0),
                                             stop=(k == KT - 1))
                            sqc = trans.tile([128, 512], dt.bfloat16,
                                             tag="lnsqc", name="lnsqc",
                                             bufs=2)
                            nc.vector.tensor_mul(sqc[:, :n],
                                                 pre_bf[k][:, o:o + n],
                                                 pre_bf[k][:, o:o + n])
                            nc.tensor.matmul(ps2[:, :n], ones_bf[:],
                                             sqc[:, :n],
                                             start=(k == 0),
                                             stop=(k == KT - 1))
                        nc.vector.tensor_scalar_mul(srow[:, o:o + n],
                                                    ps[:, :n], -1.0 / D)
                        nc.vector.tensor_scalar_mul(qrow[:, o:o + n],
                                                    ps2[:, :n], 1.0 / D)
                m2 = trans.tile([1, NEXT], dt.float32, tag="ln_m2",
                                name="ln_m2", bufs=1)
                rst = trans.tile([1, NEXT], dt.float32, tag="ln_rst",
                                name="ln_rst", bufs=1)
                nc.vector.tensor_mul(m2[:, :ncols], srow[:, :ncols],
                                     srow[:, :ncols])
                nc.vector.tensor_sub(qrow[:, :ncols], qrow[:, :ncols],
                                     m2[:, :ncols])
                nc.scalar.activation(m2[:, :ncols], qrow[:, :ncols],
                                     AF.Sqrt, bias=eps_t[:])
                nc.vector.reciprocal(rst[:, :ncols], m2[:, :ncols])
                nm_b = trans.tile([128, NEXT], dt.float32, tag="ln_nmb",
                                  name="ln_nmb", bufs=1)
                rs_b = trans.tile([128, NEXT], dt.float32, tag="ln_rsb",
                                  name="ln_rsb", bufs=1)
                nc.gpsimd.partition_broadcast(nm_b[:, :ncols], srow[:, :ncols])
                nc.gpsimd.partition_broadcast(rs_b[:, :ncols], rst[:, :ncols])
                out_f32, out_bf = [], []
                for k in range(KT):
                    xc = trans.tile([128, NEXT], dt.float32, tag="ln_xc",
                                    name="ln_xc", bufs=1)
                    nc.vector.tensor_add(xc[:, :ncols], xf32[k][:],
                                         nm_b[:, :ncols])
                    nc.vector.tensor_mul(xc[:, :ncols], xc[:, :ncols],
                                         rs_b[:, :ncols])
                    of = perm.tile([128, ncols], dt.float32, tag=out_tags[k],
                                   name=out_tags[k])
                    nc.vector.tensor_scalar(of[:], xc[:, :ncols], gam[k][:],
                                            bet[k][:], ALU.mult, ALU.add)
                    out_f32.append(of)
                    if want_bf:
                        ob = perm.tile([128, ncols], dt.bfloat16,
                                       tag=bf_tags[k], name=bf_tags[k])
                        nc.vector.tensor_copy(ob[:], of[:])
                        out_bf.append(ob)
                return out_f32, out_bf

            # ---------------- phase A: embeddings + emb LN -------------------
            x0T, x0T_bf = ln_feat(xpre, NEXT, g_emb_t, b_emb_t,
                                  [f"xz{k}" for k in range(KT)],
                                  True, [f"preb{k}" for k in range(KT)])
            for k in range(KT):
                nc.sync.dma_start(x0row_out[k], x0T[k][:, EXT:EXT + 1])

            # ---------------- phase B: projections ---------------------------
            def proj_fm(col0, ncols, bias_ts, tag, psm, wtiles):
                outs = []
                nspl = [(o, min(512, ncols - o)) for o in range(0, ncols, 512)]
                for m in range(KT):
                    out = perm.tile([128, ncols], dt.bfloat16, tag=f"{tag}{m}",
                                    name=f"{tag}{m}")
                    for (o, n) in nspl:
                        ps = psm.tile([128, 512], dt.float32, tag="projps",
                                      name="projps")
                        for k in range(KT):
                            nc.tensor.matmul(
                                ps[:, :n],
                                wtiles[k][:, m * 128:(m + 1) * 128],
                                x0T_bf[k][:, col0 + o:col0 + o + n],
                                start=(k == 0), stop=(k == KT - 1))
                        if bias_ts is None:
                            nc.vector.tensor_copy(out[:, o:o + n], ps[:, :n])
                        else:
                            nc.scalar.activation(out[:, o:o + n], ps[:, :n],
                                                 AF.Identity,
                                                 bias=bias_ts[m][:])
                    outs.append(out)
                return outs

            with tc.tile_pool(name="psB", bufs=4, space="PSUM") as psB:
                Wk_t = load_w_shared(Wk, KT, D)
                KT_bf = proj_fm(0, NEXT, None, "ad", psB, Wk_t)
                Wq_t = load_w_shared(Wq, KT, D)
                QT_bf = proj_fm(C, OWN, bq_t, "QTb", psB, Wq_t)
                Wkg_t = load_w_shared(Wkg, KT, D)
                kgT_bf = proj_fm(C, OWN, None, "kgTb", psB, Wkg_t)
                Wqg_t = load_w_shared(Wqg, KT, D)
                qgT_bf = proj_fm(EXT, 1, bqg_t, "qgTb", psB, Wqg_t)

            with tc.tile_pool(name="psV", bufs=1, space="PSUM") as psV:
                Wv_t = load_w_shared(Wv, KT, D)
                V_bf = []
                for t in range(8):
                    ps = psV.tile([128, D], dt.float32, tag="vps",
                                  name="vps", bufs=2)
                    for (o, n) in [(0, 512), (512, 256)]:
                        for k in range(KT):
                            nc.tensor.matmul(
                                ps[:, o:o + n],
                                x0T_bf[k][:, t * 128:(t + 1) * 128],
                                Wv_t[k][:, o:o + n],
                                start=(k == 0), stop=(k == KT - 1))
                    vb = perm.tile([128, H, 65], dt.bfloat16, tag=f"Vb{t}",
                                   name=f"Vb{t}")
                    for h in range(H):
                        nc.vector.tensor_copy(vb[:, h, 0:64],
                                              ps[:, h * 64:(h + 1) * 64])
                    nc.vector.memset(vb[:, :, 64:65], 1.0)
                    V_bf.append(vb)
                v0_bf = perm.tile([1, H, 65], dt.bfloat16, tag="v0b",
                                  name="v0b")
                ps0 = psV.tile([1, D], dt.float32, tag="v0ps", name="v0ps",
                               bufs=1)
                for (o, n) in [(0, 512), (512, 256)]:
                    for k in range(KT):
                        nc.tensor.matmul(ps0[:, o:o + n],
                                         x0T_bf[k][:, EXT:EXT + 1],
                                         Wv_t[k][:, o:o + n],
                                         start=(k == 0), stop=(k == KT - 1))
                for h in range(H):
                    nc.vector.tensor_copy(v0_bf[:, h, 0:64],
                                          ps0[:, h * 64:(h + 1) * 64])
                nc.vector.memset(v0_bf[:, :, 64:65], 1.0)
                Wvg_t = load_w_shared(Wvg, KT, D)
                vg_bf = []
                for t in range(4):
                    ps = psV.tile([128, D], dt.float32, tag="vps",
                                  name="vps", bufs=2)
                    for (o, n) in [(0, 512), (512, 256)]:
                        for k in range(KT):
                            nc.tensor.matmul(
                                ps[:, o:o + n],
                                x0T_bf[k][:, C + t * 128:C + (t + 1) * 128],
                                Wvg_t[k][:, o:o + n],
                                start=(k == 0), stop=(k == KT - 1))
                    vb = perm.tile([128, D], dt.bfloat16, tag=f"vgb{t}",
                                   name=f"vgb{t}")
                    nc.vector.tensor_copy(vb[:], ps[:])
                    vg_bf.append(vb)

            mk_t = []
            for lt in range(2):
                mt_ = perm.tile([128, 6, 256], dt.bfloat16,
                                tag=f"mk{lt}", name=f"mk{lt}")
                for i in range(6):
                    nc.sync.dma_start(mt_[:, i, :], maskT[lt, i])
                mk_t.append(mt_)

            # ---------------- phase F: layer-1 global stats ------------------
            gs_sb = perm.tile([65, H], dt.float32, tag="gs", name="gs")
            with tc.tile_pool(name="psF", bufs=2, space="PSUM") as psF:
                for pt in range(KT):
                    hA, hB = 2 * pt, 2 * pt + 1
                    qg2c = trans.tile([128, 2], dt.bfloat16, tag="qg2c",
                                      name="qg2c", bufs=2)
                    nc.vector.memset(qg2c[:], 0.0)
                    nc.vector.tensor_copy(qg2c[0:64, 0:1], qgT_bf[pt][0:64, :])
                    nc.vector.tensor_copy(qg2c[64:128, 1:2],
                                          qgT_bf[pt][64:128, :])
                    ps_acc = psF.tile([128, 2], dt.float32, tag="facc",
                                      name="facc")
                    ps_sum = psF.tile([1, 2], dt.float32, tag="fsum",
                                      name="fsum")
                    for j in range(4):
                        ps_s = psF.tile([128, 2], dt.float32, tag="fsgf",
                                        name="fsgf")
                        nc.tensor.matmul(
                            ps_s[:],
                            kgT_bf[pt][:, j * 128:(j + 1) * 128], qg2c[:])
                        e = trans.tile([128, 2], dt.float32, tag="fe",
                                       name="fe")
                        nc.scalar.activation(e[:], ps_s[:], AF.Exp)
                        eb = trans.tile([128, 2], dt.bfloat16, tag="feb",
                                        name="feb")
                        nc.vector.tensor_scalar_mul(eb[:], e[:],
                                                    gmask_t[j][:])
                        nc.tensor.matmul(ps_acc[:],
                                         vg_bf[j][:, pt * 128:(pt + 1) * 128],
                                         eb[:], start=(j == 0), stop=(j == 3))
                        nc.tensor.matmul(ps_sum[:], ones_bf[:], eb[:],
                                         start=(j == 0), stop=(j == 3))
                    nc.vector.tensor_copy(gs_sb[0:64, hA:hA + 1],
                                          ps_acc[0:64, 0:1])
                    nc.vector.tensor_copy(gs_sb[0:64, hB:hB + 1],
                                          ps_acc[64:128, 1:2])
                    nc.vector.tensor_copy(gs_sb[64:65, hA:hA + 1],
                                          ps_sum[0:1, 0:1])
                    nc.vector.tensor_copy(gs_sb[64:65, hB:hB + 1],
                                          ps_sum[0:1, 1:2])
            nc.sync.dma_start(gstats_out[:], gs_sb[:])

            # ---------------- phase C: windowed attention --------------------
            attnT_bf = []
            for k in range(KT):
                at = perm.tile([128, OWN], dt.bfloat16, tag=f"at{k}",
                               name=f"at{k}")
                attnT_bf.append(at)
            with (tc.tile_pool(name="psS", bufs=1, space="PSUM") as psS,
                  tc.tile_pool(name="psG", bufs=1, space="PSUM") as psG,
                  tc.tile_pool(name="psO", bufs=2, space="PSUM") as psO):
                def stage1(h, lt):
                    pt, ro = h // 2, (h % 2) * 64
                    qs = QT_bf[pt][ro:ro + 64, lt * 256:(lt + 1) * 256]
                    es = trans.tile([128, 6, 256], dt.bfloat16, tag="es",
                                    name="es", bufs=2)
                    for w in range(2):
                        ps_s = psS.tile([128, 3, 256], dt.float32,
                                        tag="ps_s", name="ps_s", bufs=2)
                        for i3 in range(3):
                            i = w * 3 + i3
                            nc.tensor.matmul(
                                ps_s[:, i3, :],
                                KT_bf[pt][ro:ro + 64,
                                          lt * 256 + i * 128:
                                          lt * 256 + (i + 1) * 128],
                                qs)
                        nc.scalar.activation(es[:, w * 3:(w + 1) * 3, :],
                                             ps_s[:, :, :], AF.Exp)
                    nc.vector.tensor_mul(es[:, :, :], es[:, :, :],
                                         mk_t[lt][:, :, :])
                    ps_sg = psG.tile([1, 256], dt.float32, tag="ps_sg",
                                     name="ps_sg")
                    nc.tensor.matmul(ps_sg[:],
                                     KT_bf[pt][ro:ro + 64, EXT:EXT + 1], qs)
                    esg = trans.tile([1, 256], dt.bfloat16, tag="esg",
                                     name="esg", bufs=2)
                    nc.scalar.activation(esg[:], ps_sg[:], AF.Exp)
                    return es, esg

                def stage2(h, lt, es, esg):
                    pt, ro = h // 2, (h % 2) * 64
                    ps_o = psO.tile([65, 256], dt.float32, tag="ps_o",
                                    name="ps_o")
                    for i in range(6):
                        nc.tensor.matmul(
                            ps_o[:],
                            V_bf[lt * 2 + i][:, h, :],
                            es[:, i, :], start=(i == 0), stop=False)
                    nc.tensor.matmul(ps_o[:], v0_bf[:, h, :],
                                     esg[:], start=False, stop=True)
                    rrow = trans.tile([1, 256], dt.float32, tag="rrow",
                                      name="rrow", bufs=2)
                    nc.vector.reciprocal(rrow[:], ps_o[64:65, :])
                    rb = trans.tile([64, 256], dt.float32, tag="rb",
                                    name="rb", bufs=2)
                    nc.gpsimd.partition_broadcast(rb[:], rrow[:])
                    nc.vector.tensor_mul(
                        attnT_bf[pt][ro:ro + 64, lt * 256:(lt + 1) * 256],
                        ps_o[0:64, :], rb[:])

                pend = None
                for pt in range(KT):
                    for lt in range(2):
                        for h in (2 * pt, 2 * pt + 1):
                            cur = stage1(h, lt)
                            if pend is not None:
                                stage2(*pend)
                            pend = (h, lt, *cur)
                stage2(*pend)

            # ---------------- phase D: attn proj + residual + LN1 ------------
            with tc.tile_pool(name="psD", bufs=4, space="PSUM") as psD:
                Wo_t = load_w_shared(Wo, KT, D)
                apre = []
                for m in range(KT):
                    ps = psD.tile([128, OWN], dt.float32, tag="dps",
                                  name="dps")
                    for k in range(KT):
                        nc.tensor.matmul(ps[:],
                                         Wo_t[k][:, m * 128:(m + 1) * 128],
                                         attnT_bf[k][:],
                                         start=(k == 0), stop=(k == KT - 1))
                    asb = trans.tile([128, OWN], dt.float32, tag="asb",
                                     name="asb")
                    nc.scalar.activation(asb[:], ps[:], AF.Identity,
                                         bias=bo_t[m][:])
                    ad = perm.tile([128, OWN], dt.float32, tag=f"ad{m}",
                                   name=f"ad{m}")
                    nc.vector.tensor_add(ad[:], asb[:], x0T[m][:, C:C + OWN])
                    apre.append(ad)
            xmT, xmT_bf = ln_feat(apre, OWN, g1_t, be1_t,
                                  [f"ad{k}" for k in range(KT)],
                                  True, [f"Vb{k}" for k in range(KT)])

            # ---------------- phase E: FFN + residual + LN2 ------------------
            with tc.tile_pool(name="psE", bufs=1, space="PSUM") as psE:
                W1_t = load_w_shared(W1, KT, FF, big=True)
                yps = [psE.tile([128, OWN], dt.float32, tag=f"yps{m}",
                                name=f"yps{m}") for m in range(KT)]
                ypre = []
                for m in range(MT_FF):
                    ps = psE.tile([128, OWN], dt.float32, tag="w1ps",
                                  name="w1ps", bufs=2)
                    for k in range(KT):
                        nc.tensor.matmul(ps[:],
                                         W1_t[k][:, m * 128:(m + 1) * 128],
                                         xmT_bf[k][:],
                                         start=(k == 0), stop=(k == KT - 1))
                    ht = trans.tile([128, OWN], dt.bfloat16, tag="hT",
                                    name="hT", bufs=3)
                    nc.scalar.activation(ht[:], ps[:], AF.Gelu_apprx_tanh,
                                         bias=b1_t[m][:])
                    w2t = trans.tile([128, D], dt.bfloat16, tag="w2ld",
                                     name="w2ld", bufs=3)
                    nc.sync.dma_start(w2t[:], W2[m])
                    for mo in range(KT):
                        nc.tensor.matmul(yps[mo][:],
                                         w2t[:, mo * 128:(mo + 1) * 128],
                                         ht[:], start=(m == 0),
                                         stop=(m == MT_FF - 1))
                for m in range(KT):
                    ysb = trans.tile([128, OWN], dt.float32, tag="ysb",
                                     name="ysb")
                    nc.scalar.activation(ysb[:], yps[m][:], AF.Identity,
                                         bias=b2_t[m][:])
                    yz = perm.tile([128, OWN], dt.float32, tag=f"xz{m}",
                                   name=f"xz{m}")
                    nc.vector.tensor_add(yz[:], ysb[:], xmT[m][:])
                    ypre.append(yz)
            x1T, _ = ln_feat(ypre, OWN, g2_t, be2_t,
                             [f"xz{k}" for k in range(KT)], False, None)
            for k in range(KT):
                nc.sync.dma_start(x1T_out[k], x1T[k][:])

    nc.compile()
    return nc


def _build_launch2():
    import concourse.bacc as bacc
    import concourse.mybir as mybir
    import concourse.tile as tile

    dt = mybir.dt
    AF = mybir.ActivationFunctionType
    nc = bacc.Bacc("TRN2", target_bir_lowering=False, debug=False,
                   num_devices=NC_CORES)

    def din(name, shape, dtype=dt.float32):
        return nc.dram_tensor(name, shape, dtype, kind="ExternalInput").ap()

    x1T = din("x1T", [KT, 128, OWN], dt.bfloat16)
    Wkg = din("Wkg", [KT, 128, D], dt.bfloat16)
    Wvg = din("Wvg", [KT, 128, D], dt.bfloat16)
    qgT = din("qgT", [128, KT, 2], dt.bfloat16)   # zero-padded head-pair cols
    gmask = din("gmask", [128, 4])
    gstats_out = nc.dram_tensor("gstats", [65, H], dt.float32,
                                kind="ExternalOutput").ap()

    with tile.TileContext(nc) as tc:
        import contextlib
        with contextlib.ExitStack() as ctx:
            pool = ctx.enter_context(tc.tile_pool(name="pool", bufs=1))
            const = ctx.enter_context(tc.tile_pool(name="const", bufs=1))
            ones_bf = const.tile([128, 1], dt.bfloat16, tag="ones_bf",
                                 name="ones_bf")
            nc.vector.memset(ones_bf[:], 1.0)
            x1_t, Wkg_t, Wvg_t = [], [], []
            for k in range(KT):
                t = pool.tile([128, OWN], dt.bfloat16, tag=f"x1{k}",
                              name=f"x1{k}")
                nc.sync.dma_start(t[:], x1T[k])
                x1_t.append(t)
                t = pool.tile([128, D], dt.bfloat16, tag=f"Wkg{k}",
                              name=f"Wkg{k}")
                nc.sync.dma_start(t[:], Wkg[k])
                Wkg_t.append(t)
            qgt = pool.tile([128, KT, 2], dt.bfloat16, tag="qgt", name="qgt")
            nc.sync.dma_start(qgt[:], qgT[:])
            qg_t = [qgt[:, k, :] for k in range(KT)]
            gmt = pool.tile([128, 4], dt.float32, tag="gmt", name="gmt")
            nc.sync.dma_start(gmt[:], gmask[:])
            gm_t = [gmt[:, j:j + 1] for j in range(4)]
            for k in range(KT):
                t = pool.tile([128, D], dt.bfloat16, tag=f"Wvg{k}",
                              name=f"Wvg{k}")
                nc.sync.dma_start(t[:], Wvg[k])
                Wvg_t.append(t)

            with tc.tile_pool(name="ps", bufs=1, space="PSUM") as psm:
                kgT_bf = []
                for m in range(KT):
                    ps = psm.tile([128, OWN], dt.float32, tag="kps",
                                  name="kps", bufs=2)
                    for k in range(KT):
                        nc.tensor.matmul(ps[:],
                                         Wkg_t[k][:, m * 128:(m + 1) * 128],
                                         x1_t[k][:],
                                         start=(k == 0), stop=(k == KT - 1))
                    kg = pool.tile([128, OWN], dt.bfloat16, tag=f"kg{m}",
                                   name=f"kg{m}")
                    nc.vector.tensor_copy(kg[:], ps[:])
                    kgT_bf.append(kg)
                vg_bf = []
                for t in range(4):
                    ps = psm.tile([128, D], dt.float32, tag="vps",
                                  name="vps", bufs=1)
                    for (o, n) in [(0, 512), (512, 256)]:
                        for k in range(KT):
                            nc.tensor.matmul(
                                ps[:, o:o + n],
                                x1_t[k][:, t * 128:(t + 1) * 128],
                                Wvg_t[k][:, o:o + n],
                                start=(k == 0), stop=(k == KT - 1))
                    vb = pool.tile([128, D], dt.bfloat16, tag=f"vg{t}",
                                   name=f"vg{t}")
                    nc.vector.tensor_copy(vb[:], ps[:])
                    vg_bf.append(vb)
                gs_sb = pool.tile([65, H], dt.float32, tag="gs", name="gs")
                for pt in range(KT):
                    hA, hB = 2 * pt, 2 * pt + 1
                    ps_acc = psm.tile([128, 2], dt.float32, tag="facc",
                                      name="facc", bufs=1)
                    ps_sum = psm.tile([1, 2], dt.float32, tag="fsum",
                                      name="fsum", bufs=1)
                    for j in range(4):
                        ps_s = psm.tile([128, 2], dt.float32, tag="fsgf",
                                        name="fsgf", bufs=1)
                        nc.tensor.matmul(
                            ps_s[:],
                            kgT_bf[pt][:, j * 128:(j + 1) * 128], qg_t[pt][:])
                        e = pool.tile([128, 2], dt.float32, tag="fe",
                                      name="fe", bufs=3)
                        nc.scalar.activation(e[:], ps_s[:], AF.Exp)
                        eb = pool.tile([128, 2], dt.bfloat16, tag="feb",
                                       name="feb", bufs=3)
                        nc.vector.tensor_scalar_mul(eb[:], e[:], gm_t[j][:])
                        nc.tensor.matmul(ps_acc[:],
                                         vg_bf[j][:, pt * 128:(pt + 1) * 128],
                                         eb[:], start=(j == 0), stop=(j == 3))
                        nc.tensor.matmul(ps_sum[:], ones_bf[:], eb[:],
                                         start=(j == 0), stop=(j == 3))
                    nc.vector.tensor_copy(gs_sb[0:64, hA:hA + 1],
                                          ps_acc[0:64, 0:1])
                    nc.vector.tensor_copy(gs_sb[0:64, hB:hB + 1],
                                          ps_acc[64:128, 1:2])
                    nc.vector.tensor_copy(gs_sb[64:65, hA:hA + 1],
                                          ps_sum[0:1, 0:1])
                    nc.vector.tensor_copy(gs_sb[64:65, hB:hB + 1],
                                          ps_sum[0:1, 1:2])
                nc.sync.dma_start(gstats_out[:], gs_sb[:])

    nc.compile()
    return nc


# ----------------------------------------------------------------- host math
def _ln_np(x, gamma, beta, eps=1e-5):
    m = x.mean(-1, keepdims=True)
    v = ((x - m) ** 2).mean(-1, keepdims=True)
    return (x - m) / np.sqrt(v + eps) * gamma + beta


def _gelu_tanh(x):
    return 0.5 * x * (1.0 + np.tanh(np.sqrt(2 / np.pi) * (x + 0.044715 * x ** 3)))


def _row_update(x_prev, out0, p):
    a = out0 @ p["Wo"] + p["bo"]
    x = _ln_np(x_prev + a, p["ln1"][0], p["ln1"][1])
    h = _gelu_tanh(x @ p["W1"] + p["b1"])
    return _ln_np(x + h @ p["W2"] + p["b2"], p["ln2"][0], p["ln2"][1])


def _np_params(params):
    out = {}
    for k, v in params.items():
        if isinstance(v, dict):
            out[k] = _np_params(v)
        elif isinstance(v, (list, tuple)):
            out[k] = [_np_params(x) if isinstance(x, dict)
                      else np.asarray(x, np.float32) for x in v]
        else:
            out[k] = np.asarray(v, np.float32)
    return out


def _wtiles(w, ktiles):
    return np.ascontiguousarray(
        np.asarray(w, np.float32).reshape(ktiles, 128, -1)).astype(BF)


def _qg_cols(qg):
    r = qg.reshape(KT, 128)
    out = np.zeros((128, KT, 2), np.float32)
    out[0:64, :, 0] = r[:, 0:64].T
    out[64:128, :, 1] = r[:, 64:128].T
    return np.ascontiguousarray(out).astype(BF)


def _cols(b, ktiles):
    return np.ascontiguousarray(
        np.asarray(b, np.float32).reshape(ktiles, 128, 1))


def _run_retry(nc, in_maps, **kw):
    import time
    from concourse.bass_utils import run_bass_kernel_spmd
    last = None
    for attempt in range(3):
        try:
            return run_bass_kernel_spmd(nc, in_maps,
                                        core_ids=list(range(NC_CORES)), **kw)
        except Exception as e:  # transient NRT_EXEC_UNIT_UNRECOVERABLE etc.
            last = e
            time.sleep(2.0)
    raise last


def kernel(input_ids, attention_mask, params):

    ids = np.asarray(input_ids).astype(np.int64)[0]
    amask = np.asarray(attention_mask).astype(np.int32)[0]
    P = _np_params(params)
    p1, p2 = P["layers"][0], P["layers"][1]
    tok_emb, pos_emb = P["tok_emb"], P["pos_emb"]

    if "nc1" not in _cache:
        _cache["nc1"] = _build_launch1()
        _cache["nc2"] = _build_launch2()
    nc1, nc2 = _cache["nc1"], _cache["nc2"]

    # ---- per-core launch-1 inputs
    shared = {
        "Wq": _wtiles(p1["Wq"] * SCALE, KT),
        "Wk": _wtiles(p1["Wk"], KT), "Wv": _wtiles(p1["Wv"], KT),
        "Wkg": _wtiles(p1["Wkg"], KT), "Wvg": _wtiles(p1["Wvg"], KT),
        "Wqg": _wtiles(p1["Wqg"] * SCALE, KT),
        "Wo": _wtiles(p1["Wo"], KT),
        "W1": _wtiles(p1["W1"], KT),
        "W2": _wtiles(p1["W2"], MT_FF),
    }
    sm_base = np.stack([
        *(p1["bq"] * SCALE).reshape(KT, 128),
        *(p1["bqg"] * SCALE).reshape(KT, 128),
        *(p1["bo"] + p1["bv"] @ p1["Wo"]).reshape(KT, 128),
        *p1["b1"].reshape(MT_FF, 128),
        *p1["b2"].reshape(KT, 128),
        *P["emb_ln"][0].reshape(KT, 128), *P["emb_ln"][1].reshape(KT, 128),
        *p1["ln1"][0].reshape(KT, 128), *p1["ln1"][1].reshape(KT, 128),
        *p1["ln2"][0].reshape(KT, 128), *p1["ln2"][1].reshape(KT, 128),
    ], axis=1)  # (128, 84)
    in_maps = []
    for c in range(NC_CORES):
        start = c * OWN
        gpos = np.arange(start - C, start + OWN + C)
        gposc = np.clip(gpos, 0, S - 1)
        ok = ((gpos >= 0) & (gpos < S)).astype(np.float32)[:, None]
        emb = tok_emb[ids[gposc]] * ok
        pos = pos_emb[gposc] * ok
        embT = (np.concatenate([emb, tok_emb[ids[0]][None]], 0)
                + np.concatenate([pos, pos_emb[0][None]], 0)).T
        maskT = np.zeros((2, 768, 256), np.float32)
        for lt in range(2):
            t = 2 * c + lt
            j = np.arange(768)[:, None]; qi = np.arange(256)[None, :]
            kp = (t - 1) * C + j
            valid = (np.abs(j - C - qi) <= C) & (kp >= 0) & (kp < S) & (kp != 0)
            pad_ok = amask[np.clip(kp, 0, S - 1)] > 0
            maskT[lt] = (valid & pad_ok).astype(np.float32)
        m = dict(shared)
        m["embT"] = np.ascontiguousarray(embT.reshape(KT, 128, EXT + 1))
        m["maskT"] = np.ascontiguousarray(
            maskT.reshape(2, 6, 128, 256)).astype(BF)
        gm = amask[start:start + OWN].astype(np.float32).reshape(4, 128).T
        m["smalls"] = np.ascontiguousarray(
            np.concatenate([sm_base, gm], axis=1).astype(np.float32))
        in_maps.append(m)

    _cache["in_maps_nc1"] = in_maps
    res1 = _run_retry(nc1, in_maps)

    # ---- host: reduce layer-1 global stats -> x1[0]
    accs = np.zeros((H, HD), np.float64)
    sums = np.zeros(H, np.float64)
    for c in range(NC_CORES):
        gs = res1.results[c]["gstats"].astype(np.float64)
        accs += gs[:64].T.reshape(H, HD)
        sums += gs[64]
    out0 = (accs / sums[:, None]).astype(np.float32).reshape(-1) + p1["bvg"]
    x0row = res1.results[0]["x0row"].reshape(D)
    x1_0 = _row_update(x0row, out0, p1)

    # ---- launch 2
    x1T_by_core = []
    for c in range(NC_CORES):
        x1c = res1.results[c]["x1T"].reshape(D, OWN).copy()
        if c == 0:
            x1c[:, 0] = x1_0
        x1T_by_core.append(x1c)
    qg2 = x1_0 @ (p2["Wqg"] * SCALE) + p2["bqg"] * SCALE
    shared2 = {
        "Wkg": _wtiles(p2["Wkg"], KT), "Wvg": _wtiles(p2["Wvg"], KT),
        "qgT": _qg_cols(qg2),  # (128, KT, 2)
    }
    in_maps2 = []
    for c in range(NC_CORES):
        m = dict(shared2)
        m["x1T"] = np.ascontiguousarray(
            x1T_by_core[c].reshape(KT, 128, OWN)).astype(BF)
        m["gmask"] = np.ascontiguousarray(
            amask[c * OWN:(c + 1) * OWN].astype(np.float32)
            .reshape(4, 128).T)
        in_maps2.append(m)
    _cache["in_maps_nc2"] = in_maps2
    res2 = _run_retry(nc2, in_maps2)

    accs2 = np.zeros((H, HD), np.float64)
    sums2 = np.zeros(H, np.float64)
    for c in range(NC_CORES):
        gs = res2.results[c]["gstats"].astype(np.float64)
        accs2 += gs[:64].T.reshape(H, HD)
        sums2 += gs[64]
    out0_2 = (accs2 / sums2[:, None]).astype(np.float32).reshape(-1) + p2["bvg"]
    x2_0 = _row_update(x1_0, out0_2, p2)
    logits = x2_0 @ P["clf_W"] + P["clf_b"]
    return logits[None, :].astype(np.float32)


# revision 34
# speedup vs baseline: 1.2602x; 1.0208x over previous
"""Trainium2 Bass kernel for nn_LongformerClassifier (sparse_attention).

Strategy (validated against the reference in numpy first):
  - The model output is only the CLS-token logits (1, 50), so layer 2
    collapses to: global attention of token 0 over all x1 + one-row FFN.
  - Launch 1 (8 cores, sequence-parallel, 512 own tokens + 256 halo each
    side): embeddings + LN, full layer-1 (sliding-window + global-column
    attention, FFN), plus per-core partial softmax stats for layer-1
    global attention of token 0.
  - Host: reduce the tiny per-core stats -> x1[0] row (couple of matvecs).
  - Launch 2 (8 cores): per-core partial stats for layer-2 global
    attention of token 0 over x1.
  - Host: reduce -> layer-2 row FFN -> classifier -> logits.

  All activations are kept feature-major ([feature partitions, token
  free dim]) so no on-device transposes are needed anywhere; softmax is
  computed key-major without max-subtraction (scores are O(1) here) and
  partition-dim reductions are done with ones-vector matmuls.
  Matmuls run in bf16 (fp32 PSUM accumulation).
"""
import sys
import numpy as np
import ml_dtypes

sys.path.insert(0, "/opt/trn_rl_repo")

V, S, D, H, NUM_LABELS = 50265, 4096, 768, 12, 50
HD, C, FF = 64, 256, 3072
NC_CORES = 8
OWN = S // NC_CORES        # 512
EXT = OWN + 2 * C          # 1024 (token 0 appended as column EXT)
SCALE = 1.0 / np.sqrt(HD)
KT = D // 128              # 6 feature partition-tiles
MT_FF = FF // 128          # 24
BF = ml_dtypes.bfloat16

_cache = {}


# ----------------------------------------------------------------- bass build
def _mk(nc_mod, bacc_mod):
    pass


def _build_launch1():
    import concourse.bacc as bacc
    import concourse.mybir as mybir
    import concourse.tile as tile

    dt = mybir.dt
    AF = mybir.ActivationFunctionType
    ALU = mybir.AluOpType
    nc = bacc.Bacc("TRN2", target_bir_lowering=False, debug=False,
                   num_devices=NC_CORES)

    def din(name, shape, dtype=dt.float32):
        return nc.dram_tensor(name, shape, dtype, kind="ExternalInput").ap()

    def dout(name, shape, dtype=dt.float32):
        return nc.dram_tensor(name, shape, dtype, kind="ExternalOutput").ap()

    NEXT = EXT + 1  # 1025
    embT = din("embT", [KT, 128, NEXT])
    Wq = din("Wq", [KT, 128, D], dt.bfloat16)
    Wk = din("Wk", [KT, 128, D], dt.bfloat16)
    Wv = din("Wv", [KT, 128, D], dt.bfloat16)
    Wkg = din("Wkg", [KT, 128, D], dt.bfloat16)
    Wvg = din("Wvg", [KT, 128, D], dt.bfloat16)
    Wqg = din("Wqg", [KT, 128, D], dt.bfloat16)
    Wo = din("Wo", [KT, 128, D], dt.bfloat16)
    W1 = din("W1", [KT, 128, FF], dt.bfloat16)
    W2 = din("W2", [MT_FF, 128, D], dt.bfloat16)
    maskT = din("maskT", [2, 6, 128, 256], dt.bfloat16)
    smalls = din("smalls", [128, 88])   # packed per-partition params

    x1T_out = dout("x1T", [KT, 128, OWN])
    gstats_out = dout("gstats", [65, H])
    x0row_out = dout("x0row", [KT, 128, 1])

    with tile.TileContext(nc) as tc:
        import contextlib
        with contextlib.ExitStack() as ctx:
            const = ctx.enter_context(tc.tile_pool(name="const", bufs=1))
            ones_bf = const.tile([128, 1], dt.bfloat16, tag="ones_bf",
                                 name="ones_bf")
            nc.vector.memset(ones_bf[:], 1.0)
            eps_t = const.tile([1, 1], dt.float32, tag="eps_t", name="eps_t")
            nc.vector.memset(eps_t[:], 1e-5)

            perm = ctx.enter_context(tc.tile_pool(name="perm", bufs=1))
            trans = ctx.enter_context(tc.tile_pool(name="trans", bufs=2))
            wpool = ctx.enter_context(tc.tile_pool(name="wpool", bufs=1))

            # embeddings first so their DMAs head the queue
            xpre = []
            for k in range(KT):
                xz = perm.tile([128, NEXT], dt.float32, tag=f"xz{k}",
                               name=f"xz{k}")
                nc.sync.dma_start(xz[:, 0:513], embT[k][:, 0:513])
                nc.sync.dma_start(xz[:, 513:NEXT], embT[k][:, 513:NEXT])
                xpre.append(xz)

            sm_t = perm.tile([128, 88], dt.float32, tag="sm", name="sm")
            nc.sync.dma_start(sm_t[:], smalls[:])
            _off = [0]

            def sm_cols(n):
                o = _off[0]; _off[0] += n
                return [sm_t[:, o + i:o + i + 1] for i in range(n)]

            bq_t = sm_cols(KT); bqg_t = sm_cols(KT)
            bo_t = sm_cols(KT); b1_t = sm_cols(MT_FF)
            b2_t = sm_cols(KT)
            g_emb_t = sm_cols(KT); b_emb_t = sm_cols(KT)
            g1_t = sm_cols(KT); be1_t = sm_cols(KT)
            g2_t = sm_cols(KT); be2_t = sm_cols(KT)
            gmask_t = sm_cols(4)

            # weight slots: small double-buffered for 768-wide matrices,
            # big single-buffered only for W1
            def load_w_shared(ap, ntiles, cols, big=False):
                ts = []
                for k in range(ntiles):
                    if big:
                        t = wpool.tile([128, cols], dt.bfloat16, tag=f"wld{k}",
                                       name=f"wld{k}", bufs=1)
                    else:
                        t = wpool.tile([128, cols], dt.bfloat16, tag=f"wsm{k}",
                                       name=f"wsm{k}", bufs=2)
                    nc.sync.dma_start(t[:], ap[k])
                    ts.append(t)
                return ts

            # ------------- feature-major layernorm (in-place capable) -------
            def ln_feat(xf32, ncols, gam, bet, out_tags, want_bf, bf_tags):
                nspl = [(o, min(512, ncols - o)) for o in range(0, ncols, 512)]
                pre_bf = []
                for k in range(KT):
                    pb = perm.tile([128, ncols], dt.bfloat16, tag=f"preb{k}",
                                   name=f"preb{k}")
                    nc.vector.tensor_copy(pb[:], xf32[k][:])
                    pre_bf.append(pb)
                srow = trans.tile([1, NEXT], dt.float32, tag="ln_srow",
                                  name="ln_srow", bufs=1)
                qrow = trans.tile([1, NEXT], dt.float32, tag="ln_qrow",
                                  name="ln_qrow", bufs=1)
                with tc.tile_pool(name="lnps", bufs=2, space="PSUM") as psm:
                    for (o, n) in nspl:
                        ps = psm.tile([1, 512], dt.float32, tag="ln_ps",
                                      name="ln_ps")
                        ps2 = psm.tile([1, 512], dt.float32, tag="ln_ps2",
                                       name="ln_ps2")
                        for k in range(KT):
                            nc.tensor.matmul(ps[:, :n], ones_bf[:],
                                             pre_bf[k][:, o:o + n],
                                             start=(k == 0),
                                             stop=(k == KT - 1))
                            sqc = trans.tile([128, 512], dt.bfloat16,
                                             tag="lnsqc", name="lnsqc",
                                             bufs=2)
                            nc.vector.tensor_mul(sqc[:, :n],
                                                 pre_bf[k][:, o:o + n],
                                                 pre_bf[k][:, o:o + n])
                            nc.tensor.matmul(ps2[:, :n], ones_bf[:],
                                             sqc[:, :n],
                                             start=(k == 0),
                                             stop=(k == KT - 1))
                        nc.vector.tensor_scalar_mul(srow[:, o:o + n],
                                                    ps[:, :n], -1.0 / D)
                        nc.vector.tensor_scalar_mul(qrow[:, o:o + n],
                                                    ps2[:, :n], 1.0 / D)
                m2 = trans.tile([1, NEXT], dt.float32, tag="ln_m2",
                                name="ln_m2", bufs=1)
                rst = trans.tile([1, NEXT], dt.float32, tag="ln_rst",
                                name="ln_rst", bufs=1)
                nc.vector.tensor_mul(m2[:, :ncols], srow[:, :ncols],
                                     srow[:, :ncols])
                nc.vector.tensor_sub(qrow[:, :ncols], qrow[:, :ncols],
                                     m2[:, :ncols])
                nc.scalar.activation(m2[:, :ncols], qrow[:, :ncols],
                                     AF.Sqrt, bias=eps_t[:])
                nc.vector.reciprocal(rst[:, :ncols], m2[:, :ncols])
                nm_b = trans.tile([128, NEXT], dt.float32, tag="ln_nmb",
                                  name="ln_nmb", bufs=1)
                rs_b = trans.tile([128, NEXT], dt.float32, tag="ln_rsb",
                                  name="ln_rsb", bufs=1)
                nc.gpsimd.partition_broadcast(nm_b[:, :ncols], srow[:, :ncols])
                nc.gpsimd.partition_broadcast(rs_b[:, :ncols], rst[:, :ncols])
                out_f32, out_bf = [], []
                for k in range(KT):
                    xc = trans.tile([128, NEXT], dt.float32, tag="ln_xc",
                                    name="ln_xc", bufs=1)
                    nc.vector.tensor_add(xc[:, :ncols], xf32[k][:],
                                         nm_b[:, :ncols])
                    nc.vector.tensor_mul(xc[:, :ncols], xc[:, :ncols],
                                         rs_b[:, :ncols])
                    of = perm.tile([128, ncols], dt.float32, tag=out_tags[k],
                                   name=out_tags[k])
                    nc.vector.tensor_scalar(of[:], xc[:, :ncols], gam[k][:],
                                            bet[k][:], ALU.mult, ALU.add)
                    out_f32.append(of)
                    if want_bf:
                        ob = perm.tile([128, ncols], dt.bfloat16,
                                       tag=bf_tags[k], name=bf_tags[k])
                        nc.vector.tensor_copy(ob[:], of[:])
                        out_bf.append(ob)
                return out_f32, out_bf

            # ---------------- phase A: embeddings + emb LN -------------------
            x0T, x0T_bf = ln_feat(xpre, NEXT, g_emb_t, b_emb_t,
                                  [f"xz{k}" for k in range(KT)],
                                  True, [f"preb{k}" for k in range(KT)])
            for k in range(KT):
                nc.sync.dma_start(x0row_out[k], x0T[k][:, EXT:EXT + 1])

            # ---------------- phase B: projections ---------------------------
            def proj_fm(col0, ncols, bias_ts, tag, psm, wtiles):
                outs = []
                nspl = [(o, min(512, ncols - o)) for o in range(0, ncols, 512)]
                for m in range(KT):
                    out = perm.tile([128, ncols], dt.bfloat16, tag=f"{tag}{m}",
                                    name=f"{tag}{m}")
                    for (o, n) in nspl:
                        ps = psm.tile([128, 512], dt.float32, tag="projps",
                                      name="projps")
                        for k in range(KT):
                            nc.tensor.matmul(
                                ps[:, :n],
                                wtiles[k][:, m * 128:(m + 1) * 128],
                                x0T_bf[k][:, col0 + o:col0 + o + n],
                                start=(k == 0), stop=(k == KT - 1))
                        if bias_ts is None:
                            nc.vector.tensor_copy(out[:, o:o + n], ps[:, :n])
                        else:
                            nc.scalar.activation(out[:, o:o + n], ps[:, :n],
                                                 AF.Identity,
                                                 bias=bias_ts[m][:])
                    outs.append(out)
                return outs

            with tc.tile_pool(name="psB", bufs=4, space="PSUM") as psB:
                Wk_t = load_w_shared(Wk, KT, D)
                KT_bf = proj_fm(0, NEXT, None, "ad", psB, Wk_t)
                Wq_t = load_w_shared(Wq, KT, D)
                QT_bf = proj_fm(C, OWN, bq_t, "QTb", psB, Wq_t)
                Wkg_t = load_w_shared(Wkg, KT, D)
                kgT_bf = proj_fm(C, OWN, None, "kgTb", psB, Wkg_t)
                Wqg_t = load_w_shared(Wqg, KT, D)
                qgT_bf = proj_fm(EXT, 1, bqg_t, "qgTb", psB, Wqg_t)

            with tc.tile_pool(name="psV", bufs=1, space="PSUM") as psV:
                Wv_t = load_w_shared(Wv, KT, D)
                V_bf = []
                for t in range(8):
                    ps = psV.tile([128, D], dt.float32, tag="vps",
                                  name="vps", bufs=2)
                    for (o, n) in [(0, 512), (512, 256)]:
                        for k in range(KT):
                            nc.tensor.matmul(
                                ps[:, o:o + n],
                                x0T_bf[k][:, t * 128:(t + 1) * 128],
                                Wv_t[k][:, o:o + n],
                                start=(k == 0), stop=(k == KT - 1))
                    vb = perm.tile([128, H, 65], dt.bfloat16, tag=f"Vb{t}",
                                   name=f"Vb{t}")
                    for h in range(H):
                        nc.vector.tensor_copy(vb[:, h, 0:64],
                                              ps[:, h * 64:(h + 1) * 64])
                    nc.vector.memset(vb[:, :, 64:65], 1.0)
                    V_bf.append(vb)
                v0_bf = perm.tile([1, H, 65], dt.bfloat16, tag="v0b",
                                  name="v0b")
                ps0 = psV.tile([1, D], dt.float32, tag="v0ps", name="v0ps",
                               bufs=1)
                for (o, n) in [(0, 512), (512, 256)]:
                    for k in range(KT):
                        nc.tensor.matmul(ps0[:, o:o + n],
                                         x0T_bf[k][:, EXT:EXT + 1],
                                         Wv_t[k][:, o:o + n],
                                         start=(k == 0), stop=(k == KT - 1))
                for h in range(H):
                    nc.vector.tensor_copy(v0_bf[:, h, 0:64],
                                          ps0[:, h * 64:(h + 1) * 64])
                nc.vector.memset(v0_bf[:, :, 64:65], 1.0)
                Wvg_t = load_w_shared(Wvg, KT, D)
                vg_bf = []
                for t in range(4):
                    ps = psV.tile([128, D], dt.float32, tag="vps",
                                  name="vps", bufs=2)
                    for (o, n) in [(0, 512), (512, 256)]:
                        for k in range(KT):
                            nc.tensor.matmul(
                                ps[:, o:o + n],
                                x0T_bf[k][:, C + t * 128:C + (t + 1) * 128],
                                Wvg_t[k][:, o:o + n],
                                start=(k == 0), stop=(k == KT - 1))
                    vb = perm.tile([128, D], dt.bfloat16, tag=f"vgb{t}",
                                   name=f"vgb{t}")
                    nc.vector.tensor_copy(vb[:], ps[:])
                    vg_bf.append(vb)

            mk_t = []
            for lt in range(2):
                mt_ = perm.tile([128, 6, 256], dt.bfloat16,
                                tag=f"mk{lt}", name=f"mk{lt}")
                for i in range(6):
                    nc.sync.dma_start(mt_[:, i, :], maskT[lt, i])
                mk_t.append(mt_)

            # ---------------- phase F: layer-1 global stats ------------------
            gs_sb = perm.tile([65, H], dt.float32, tag="gs", name="gs")
            with tc.tile_pool(name="psF", bufs=2, space="PSUM") as psF:
                for pt in range(KT):
                    hA, hB = 2 * pt, 2 * pt + 1
                    qg2c = trans.tile([128, 2], dt.bfloat16, tag="qg2c",
                                      name="qg2c", bufs=2)
                    nc.vector.memset(qg2c[:], 0.0)
                    nc.vector.tensor_copy(qg2c[0:64, 0:1], qgT_bf[pt][0:64, :])
                    nc.vector.tensor_copy(qg2c[64:128, 1:2],
                                          qgT_bf[pt][64:128, :])
                    ps_acc = psF.tile([128, 2], dt.float32, tag="facc",
                                      name="facc")
                    ps_sum = psF.tile([1, 2], dt.float32, tag="fsum",
                                      name="fsum")
                    for j in range(4):
                        ps_s = psF.tile([128, 2], dt.float32, tag="fsgf",
                                        name="fsgf")
                        nc.tensor.matmul(
                            ps_s[:],
                            kgT_bf[pt][:, j * 128:(j + 1) * 128], qg2c[:])
                        e = trans.tile([128, 2], dt.float32, tag="fe",
                                       name="fe")
                        nc.scalar.activation(e[:], ps_s[:], AF.Exp)
                        eb = trans.tile([128, 2], dt.bfloat16, tag="feb",
                                        name="feb")
                        nc.vector.tensor_scalar_mul(eb[:], e[:],
                                                    gmask_t[j][:])
                        nc.tensor.matmul(ps_acc[:],
                                         vg_bf[j][:, pt * 128:(pt + 1) * 128],
                                         eb[:], start=(j == 0), stop=(j == 3))
                        nc.tensor.matmul(ps_sum[:], ones_bf[:], eb[:],
                                         start=(j == 0), stop=(j == 3))
                    nc.vector.tensor_copy(gs_sb[0:64, hA:hA + 1],
                                          ps_acc[0:64, 0:1])
                    nc.vector.tensor_copy(gs_sb[0:64, hB:hB + 1],
                                          ps_acc[64:128, 1:2])
                    nc.vector.tensor_copy(gs_sb[64:65, hA:hA + 1],
                                          ps_sum[0:1, 0:1])
                    nc.vector.tensor_copy(gs_sb[64:65, hB:hB + 1],
                                          ps_sum[0:1, 1:2])
            nc.sync.dma_start(gstats_out[:], gs_sb[:])

            # ---------------- phase C: windowed attention --------------------
            attnT_bf = []
            for k in range(KT):
                at = perm.tile([128, OWN], dt.bfloat16, tag=f"at{k}",
                               name=f"at{k}")
                attnT_bf.append(at)
            with (tc.tile_pool(name="psS", bufs=1, space="PSUM") as psS,
                  tc.tile_pool(name="psG", bufs=1, space="PSUM") as psG,
                  tc.tile_pool(name="psO", bufs=2, space="PSUM") as psO):
                def stage1(h, lt):
                    pt, ro = h // 2, (h % 2) * 64
                    qs = QT_bf[pt][ro:ro + 64, lt * 256:(lt + 1) * 256]
                    es = trans.tile([128, 6, 256], dt.bfloat16, tag="es",
                                    name="es", bufs=2)
                    for w in range(2):
                        ps_s = psS.tile([128, 3, 256], dt.float32,
                                        tag="ps_s", name="ps_s", bufs=2)
                        for i3 in range(3):
                            i = w * 3 + i3
                            nc.tensor.matmul(
                                ps_s[:, i3, :],
                                KT_bf[pt][ro:ro + 64,
                                          lt * 256 + i * 128:
                                          lt * 256 + (i + 1) * 128],
                                qs)
                        nc.scalar.activation(es[:, w * 3:(w + 1) * 3, :],
                                             ps_s[:, :, :], AF.Exp)
                    nc.vector.tensor_mul(es[:, :, :], es[:, :, :],
                                         mk_t[lt][:, :, :])
                    ps_sg = psG.tile([1, 256], dt.float32, tag="ps_sg",
                                     name="ps_sg")
                    nc.tensor.matmul(ps_sg[:],
                                     KT_bf[pt][ro:ro + 64, EXT:EXT + 1], qs)
                    esg = trans.tile([1, 256], dt.bfloat16, tag="esg",
                                     name="esg", bufs=2)
                    nc.scalar.activation(esg[:], ps_sg[:], AF.Exp)
                    return es, esg

                def stage2(h, lt, es, esg):
                    pt, ro = h // 2, (h % 2) * 64
                    ps_o = psO.tile([65, 256], dt.float32, tag="ps_o",
                                    name="ps_o", bufs=3)
                    for i in range(6):
                        nc.tensor.matmul(
                            ps_o[:],
                            V_bf[lt * 2 + i][:, h, :],
                            es[:, i, :], start=(i == 0), stop=False)
                    nc.tensor.matmul(ps_o[:], v0_bf[:, h, :],
                                     esg[:], start=False, stop=True)
                    rrow = trans.tile([1, 256], dt.float32, tag="rrow",
                                      name="rrow", bufs=2)
                    nc.vector.reciprocal(rrow[:], ps_o[64:65, :])
                    rb = trans.tile([64, 256], dt.float32, tag="rb",
                                    name="rb", bufs=2)
                    nc.gpsimd.partition_broadcast(rb[:], rrow[:])
                    nc.vector.tensor_mul(
                        attnT_bf[pt][ro:ro + 64, lt * 256:(lt + 1) * 256],
                        ps_o[0:64, :], rb[:])

                pend = None
                for pt in range(KT):
                    for lt in range(2):
                        for h in (2 * pt, 2 * pt + 1):
                            cur = stage1(h, lt)
                            if pend is not None:
                                stage2(*pend)
                            pend = (h, lt, *cur)
                stage2(*pend)

            # ---------------- phase D: attn proj + residual + LN1 ------------
            with tc.tile_pool(name="psD", bufs=4, space="PSUM") as psD:
                Wo_t = load_w_shared(Wo, KT, D)
                apre = []
                for m in range(KT):
                    ps = psD.tile([128, OWN], dt.float32, tag="dps",
                                  name="dps")
                    for k in range(KT):
                        nc.tensor.matmul(ps[:],
                                         Wo_t[k][:, m * 128:(m + 1) * 128],
                                         attnT_bf[k][:],
                                         start=(k == 0), stop=(k == KT - 1))
                    asb = trans.tile([128, OWN], dt.float32, tag="asb",
                                     name="asb")
                    nc.scalar.activation(asb[:], ps[:], AF.Identity,
                                         bias=bo_t[m][:])
                    ad = perm.tile([128, OWN], dt.float32, tag=f"ad{m}",
                                   name=f"ad{m}")
                    nc.vector.tensor_add(ad[:], asb[:], x0T[m][:, C:C + OWN])
                    apre.append(ad)
            xmT, xmT_bf = ln_feat(apre, OWN, g1_t, be1_t,
                                  [f"ad{k}" for k in range(KT)],
                                  True, [f"Vb{k}" for k in range(KT)])

            # ---------------- phase E: FFN + residual + LN2 ------------------
            with tc.tile_pool(name="psE", bufs=1, space="PSUM") as psE:
                W1_t = load_w_shared(W1, KT, FF, big=True)
                yps = [psE.tile([128, OWN], dt.float32, tag=f"yps{m}",
                                name=f"yps{m}") for m in range(KT)]
                ypre = []
                for m in range(MT_FF):
                    ps = psE.tile([128, OWN], dt.float32, tag="w1ps",
                                  name="w1ps", bufs=2)
                    for k in range(KT):
                        nc.tensor.matmul(ps[:],
                                         W1_t[k][:, m * 128:(m + 1) * 128],
                                         xmT_bf[k][:],
                                         start=(k == 0), stop=(k == KT - 1))
                    ht = trans.tile([128, OWN], dt.bfloat16, tag="hT",
                                    name="hT", bufs=3)
                    nc.scalar.activation(ht[:], ps[:], AF.Gelu_apprx_tanh,
                                         bias=b1_t[m][:])
                    w2t = trans.tile([128, D], dt.bfloat16, tag="w2ld",
                                     name="w2ld", bufs=3)
                    nc.sync.dma_start(w2t[:], W2[m])
                    for mo in range(KT):
                        nc.tensor.matmul(yps[mo][:],
                                         w2t[:, mo * 128:(mo + 1) * 128],
                                         ht[:], start=(m == 0),
                                         stop=(m == MT_FF - 1))
                for m in range(KT):
                    ysb = trans.tile([128, OWN], dt.float32, tag="ysb",
                                     name="ysb")
                    nc.scalar.activation(ysb[:], yps[m][:], AF.Identity,
                                         bias=b2_t[m][:])
                    yz = perm.tile([128, OWN], dt.float32, tag=f"xz{m}",
                                   name=f"xz{m}")
                    nc.vector.tensor_add(yz[:], ysb[:], xmT[m][:])
                    ypre.append(yz)
            x1T, _ = ln_feat(ypre, OWN, g2_t, be2_t,
                             [f"xz{k}" for k in range(KT)], False, None)
            for k in range(KT):
                nc.sync.dma_start(x1T_out[k], x1T[k][:])

    nc.compile()
    return nc


def _build_launch2():
    import concourse.bacc as bacc
    import concourse.mybir as mybir
    import concourse.tile as tile

    dt = mybir.dt
    AF = mybir.ActivationFunctionType
    nc = bacc.Bacc("TRN2", target_bir_lowering=False, debug=False,
                   num_devices=NC_CORES)

    def din(name, shape, dtype=dt.float32):
        return nc.dram_tensor(name, shape, dtype, kind="ExternalInput").ap()

    x1T = din("x1T", [KT, 128, OWN], dt.bfloat16)
    Wkg = din("Wkg", [KT, 128, D], dt.bfloat16)
    Wvg = din("Wvg", [KT, 128, D], dt.bfloat16)
    qgT = din("qgT", [128, KT, 2], dt.bfloat16)   # zero-padded head-pair cols
    gmask = din("gmask", [128, 4])
    gstats_out = nc.dram_tensor("gstats", [65, H], dt.float32,
                                kind="ExternalOutput").ap()

    with tile.TileContext(nc) as tc:
        import contextlib
        with contextlib.ExitStack() as ctx:
            pool = ctx.enter_context(tc.tile_pool(name="pool", bufs=1))
            const = ctx.enter_context(tc.tile_pool(name="const", bufs=1))
            ones_bf = const.tile([128, 1], dt.bfloat16, tag="ones_bf",
                                 name="ones_bf")
            nc.vector.memset(ones_bf[:], 1.0)
            x1_t, Wkg_t, Wvg_t = [], [], []
            for k in range(KT):
                t = pool.tile([128, OWN], dt.bfloat16, tag=f"x1{k}",
                              name=f"x1{k}")
                nc.sync.dma_start(t[:], x1T[k])
                x1_t.append(t)
                t = pool.tile([128, D], dt.bfloat16, tag=f"Wkg{k}",
                              name=f"Wkg{k}")
                nc.sync.dma_start(t[:], Wkg[k])
                Wkg_t.append(t)
            qgt = pool.tile([128, KT, 2], dt.bfloat16, tag="qgt", name="qgt")
            nc.sync.dma_start(qgt[:], qgT[:])
            qg_t = [qgt[:, k, :] for k in range(KT)]
            gmt = pool.tile([128, 4], dt.float32, tag="gmt", name="gmt")
            nc.sync.dma_start(gmt[:], gmask[:])
            gm_t = [gmt[:, j:j + 1] for j in range(4)]
            for k in range(KT):
                t = pool.tile([128, D], dt.bfloat16, tag=f"Wvg{k}",
                              name=f"Wvg{k}")
                nc.sync.dma_start(t[:], Wvg[k])
                Wvg_t.append(t)

            with tc.tile_pool(name="ps", bufs=1, space="PSUM") as psm:
                kgT_bf = []
                for m in range(KT):
                    ps = psm.tile([128, OWN], dt.float32, tag="kps",
                                  name="kps", bufs=2)
                    for k in range(KT):
                        nc.tensor.matmul(ps[:],
                                         Wkg_t[k][:, m * 128:(m + 1) * 128],
                                         x1_t[k][:],
                                         start=(k == 0), stop=(k == KT - 1))
                    kg = pool.tile([128, OWN], dt.bfloat16, tag=f"kg{m}",
                                   name=f"kg{m}")
                    nc.vector.tensor_copy(kg[:], ps[:])
                    kgT_bf.append(kg)
                vg_bf = []
                for t in range(4):
                    ps = psm.tile([128, D], dt.float32, tag="vps",
                                  name="vps", bufs=1)
                    for (o, n) in [(0, 512), (512, 256)]:
                        for k in range(KT):
                            nc.tensor.matmul(
                                ps[:, o:o + n],
                                x1_t[k][:, t * 128:(t + 1) * 128],
                                Wvg_t[k][:, o:o + n],
                                start=(k == 0), stop=(k == KT - 1))
                    vb = pool.tile([128, D], dt.bfloat16, tag=f"vg{t}",
                                   name=f"vg{t}")
                    nc.vector.tensor_copy(vb[:], ps[:])
                    vg_bf.append(vb)
                gs_sb = pool.tile([65, H], dt.float32, tag="gs", name="gs")
                for pt in range(KT):
                    hA, hB = 2 * pt, 2 * pt + 1
                    ps_acc = psm.tile([128, 2], dt.float32, tag="facc",
                                      name="facc", bufs=1)
                    ps_sum = psm.tile([1, 2], dt.float32, tag="fsum",
                                      name="fsum", bufs=1)
                    for j in range(4):
                        ps_s = psm.tile([128, 2], dt.float32, tag="fsgf",
                                        name="fsgf", bufs=1)
                        nc.tensor.matmul(
                            ps_s[:],
                            kgT_bf[pt][:, j * 128:(j + 1) * 128], qg_t[pt][:])
                        e = pool.tile([128, 2], dt.float32, tag="fe",
                                      name="fe", bufs=3)
                        nc.scalar.activation(e[:], ps_s[:], AF.Exp)
                        eb = pool.tile([128, 2], dt.bfloat16, tag="feb",
                                       name="feb", bufs=3)
                        nc.vector.tensor_scalar_mul(eb[:], e[:], gm_t[j][:])
                        nc.tensor.matmul(ps_acc[:],
                                         vg_bf[j][:, pt * 128:(pt + 1) * 128],
                                         eb[:], start=(j == 0), stop=(j == 3))
                        nc.tensor.matmul(ps_sum[:], ones_bf[:], eb[:],
                                         start=(j == 0), stop=(j == 3))
                    nc.vector.tensor_copy(gs_sb[0:64, hA:hA + 1],
                                          ps_acc[0:64, 0:1])
                    nc.vector.tensor_copy(gs_sb[0:64, hB:hB + 1],
                                          ps_acc[64:128, 1:2])
                    nc.vector.tensor_copy(gs_sb[64:65, hA:hA + 1],
                                          ps_sum[0:1, 0:1])
                    nc.vector.tensor_copy(gs_sb[64:65, hB:hB + 1],
                                          ps_sum[0:1, 1:2])
                nc.sync.dma_start(gstats_out[:], gs_sb[:])

    nc.compile()
    return nc


# ----------------------------------------------------------------- host math
def _ln_np(x, gamma, beta, eps=1e-5):
    m = x.mean(-1, keepdims=True)
    v = ((x - m) ** 2).mean(-1, keepdims=True)
    return (x - m) / np.sqrt(v + eps) * gamma + beta


def _gelu_tanh(x):
    return 0.5 * x * (1.0 + np.tanh(np.sqrt(2 / np.pi) * (x + 0.044715 * x ** 3)))


def _row_update(x_prev, out0, p):
    a = out0 @ p["Wo"] + p["bo"]
    x = _ln_np(x_prev + a, p["ln1"][0], p["ln1"][1])
    h = _gelu_tanh(x @ p["W1"] + p["b1"])
    return _ln_np(x + h @ p["W2"] + p["b2"], p["ln2"][0], p["ln2"][1])


def _np_params(params):
    out = {}
    for k, v in params.items():
        if isinstance(v, dict):
            out[k] = _np_params(v)
        elif isinstance(v, (list, tuple)):
            out[k] = [_np_params(x) if isinstance(x, dict)
                      else np.asarray(x, np.float32) for x in v]
        else:
            out[k] = np.asarray(v, np.float32)
    return out


def _wtiles(w, ktiles):
    return np.ascontiguousarray(
        np.asarray(w, np.float32).reshape(ktiles, 128, -1)).astype(BF)


def _qg_cols(qg):
    r = qg.reshape(KT, 128)
    out = np.zeros((128, KT, 2), np.float32)
    out[0:64, :, 0] = r[:, 0:64].T
    out[64:128, :, 1] = r[:, 64:128].T
    return np.ascontiguousarray(out).astype(BF)


def _cols(b, ktiles):
    return np.ascontiguousarray(
        np.asarray(b, np.float32).reshape(ktiles, 128, 1))


def _run_retry(nc, in_maps, **kw):
    import time
    from concourse.bass_utils import run_bass_kernel_spmd
    last = None
    for attempt in range(3):
        try:
            return run_bass_kernel_spmd(nc, in_maps,
                                        core_ids=list(range(NC_CORES)), **kw)
        except Exception as e:  # transient NRT_EXEC_UNIT_UNRECOVERABLE etc.
            last = e
            time.sleep(2.0)
    raise last


def kernel(input_ids, attention_mask, params):

    ids = np.asarray(input_ids).astype(np.int64)[0]
    amask = np.asarray(attention_mask).astype(np.int32)[0]
    P = _np_params(params)
    p1, p2 = P["layers"][0], P["layers"][1]
    tok_emb, pos_emb = P["tok_emb"], P["pos_emb"]

    if "nc1" not in _cache:
        _cache["nc1"] = _build_launch1()
        _cache["nc2"] = _build_launch2()
    nc1, nc2 = _cache["nc1"], _cache["nc2"]

    # ---- per-core launch-1 inputs
    shared = {
        "Wq": _wtiles(p1["Wq"] * SCALE, KT),
        "Wk": _wtiles(p1["Wk"], KT), "Wv": _wtiles(p1["Wv"], KT),
        "Wkg": _wtiles(p1["Wkg"], KT), "Wvg": _wtiles(p1["Wvg"], KT),
        "Wqg": _wtiles(p1["Wqg"] * SCALE, KT),
        "Wo": _wtiles(p1["Wo"], KT),
        "W1": _wtiles(p1["W1"], KT),
        "W2": _wtiles(p1["W2"], MT_FF),
    }
    sm_base = np.stack([
        *(p1["bq"] * SCALE).reshape(KT, 128),
        *(p1["bqg"] * SCALE).reshape(KT, 128),
        *(p1["bo"] + p1["bv"] @ p1["Wo"]).reshape(KT, 128),
        *p1["b1"].reshape(MT_FF, 128),
        *p1["b2"].reshape(KT, 128),
        *P["emb_ln"][0].reshape(KT, 128), *P["emb_ln"][1].reshape(KT, 128),
        *p1["ln1"][0].reshape(KT, 128), *p1["ln1"][1].reshape(KT, 128),
        *p1["ln2"][0].reshape(KT, 128), *p1["ln2"][1].reshape(KT, 128),
    ], axis=1)  # (128, 84)
    in_maps = []
    for c in range(NC_CORES):
        start = c * OWN
        gpos = np.arange(start - C, start + OWN + C)
        gposc = np.clip(gpos, 0, S - 1)
        ok = ((gpos >= 0) & (gpos < S)).astype(np.float32)[:, None]
        emb = tok_emb[ids[gposc]] * ok
        pos = pos_emb[gposc] * ok
        embT = (np.concatenate([emb, tok_emb[ids[0]][None]], 0)
                + np.concatenate([pos, pos_emb[0][None]], 0)).T
        maskT = np.zeros((2, 768, 256), np.float32)
        for lt in range(2):
            t = 2 * c + lt
            j = np.arange(768)[:, None]; qi = np.arange(256)[None, :]
            kp = (t - 1) * C + j
            valid = (np.abs(j - C - qi) <= C) & (kp >= 0) & (kp < S) & (kp != 0)
            pad_ok = amask[np.clip(kp, 0, S - 1)] > 0
            maskT[lt] = (valid & pad_ok).astype(np.float32)
        m = dict(shared)
        m["embT"] = np.ascontiguousarray(embT.reshape(KT, 128, EXT + 1))
        m["maskT"] = np.ascontiguousarray(
            maskT.reshape(2, 6, 128, 256)).astype(BF)
        gm = amask[start:start + OWN].astype(np.float32).reshape(4, 128).T
        m["smalls"] = np.ascontiguousarray(
            np.concatenate([sm_base, gm], axis=1).astype(np.float32))
        in_maps.append(m)

    _cache["in_maps_nc1"] = in_maps
    res1 = _run_retry(nc1, in_maps)

    # ---- host: reduce layer-1 global stats -> x1[0]
    accs = np.zeros((H, HD), np.float64)
    sums = np.zeros(H, np.float64)
    for c in range(NC_CORES):
        gs = res1.results[c]["gstats"].astype(np.float64)
        accs += gs[:64].T.reshape(H, HD)
        sums += gs[64]
    out0 = (accs / sums[:, None]).astype(np.float32).reshape(-1) + p1["bvg"]
    x0row = res1.results[0]["x0row"].reshape(D)
    x1_0 = _row_update(x0row, out0, p1)

    # ---- launch 2
    x1T_by_core = []
    for c in range(NC_CORES):
        x1c = res1.results[c]["x1T"].reshape(D, OWN).copy()
        if c == 0:
            x1c[:, 0] = x1_0
        x1T_by_core.append(x1c)
    qg2 = x1_0 @ (p2["Wqg"] * SCALE) + p2["bqg"] * SCALE
    shared2 = {
        "Wkg": _wtiles(p2["Wkg"], KT), "Wvg": _wtiles(p2["Wvg"], KT),
        "qgT": _qg_cols(qg2),  # (128, KT, 2)
    }
    in_maps2 = []
    for c in range(NC_CORES):
        m = dict(shared2)
        m["x1T"] = np.ascontiguousarray(
            x1T_by_core[c].reshape(KT, 128, OWN)).astype(BF)
        m["gmask"] = np.ascontiguousarray(
            amask[c * OWN:(c + 1) * OWN].astype(np.float32)
            .reshape(4, 128).T)
        in_maps2.append(m)
    _cache["in_maps_nc2"] = in_maps2
    res2 = _run_retry(nc2, in_maps2)

    accs2 = np.zeros((H, HD), np.float64)
    sums2 = np.zeros(H, np.float64)
    for c in range(NC_CORES):
        gs = res2.results[c]["gstats"].astype(np.float64)
        accs2 += gs[:64].T.reshape(H, HD)
        sums2 += gs[64]
    out0_2 = (accs2 / sums2[:, None]).astype(np.float32).reshape(-1) + p2["bvg"]
    x2_0 = _row_update(x1_0, out0_2, p2)
    logits = x2_0 @ P["clf_W"] + P["clf_b"]
    return logits[None, :].astype(np.float32)
